# revision 1
# baseline (speedup 1.0000x reference)
"""Trainium2 Bass kernel for MultiScaleChannelTransformerBlock.

kernel(**inputs) takes the FULL inputs (as produced by setup_inputs())
and returns the FULL output [2, 128, 256, 256] float32.

Sharding: spatial over H across 8 NeuronCores (32 rows each, plus a
1-row halo on each side, host-padded).  The only cross-core
communication is an AllReduce of the per-(batch,scale) attention Gram
matrices and q/k squared norms (tiny).

Per-core layout: channels on SBUF partitions (C=128), pixels on the
free dimension.  Big matmuls run in float32r (fp22 multiply, fp32
accumulate).  The 3x3 depthwise conv of the FFN is folded into the wpi
1x1 conv: 9 PSUM-accumulated matmuls with host-precomputed weights
wpi[o,j]*wdw[o,dy,dx] against shifted views of z (z carries one zero
column of padding each side; halo rows are zeroed via a per-core mask
so image-boundary taps contribute nothing).
"""

import numpy as np

B = 2
C = 128
CO = 32
HID = 340
W = 256
SCALES = [1, 2, 4, 8]
NS = len(SCALES)
NU = NS * B
NTAPS = sum(r * r for r in SCALES)  # 85
EPS = 1e-5

_CACHE = {}


def _build(nrows, n_cores, debug=False):
    """Build + compile the SPMD device program for one core holding
    `nrows` exclusive image rows (plus 1 halo row each side)."""
    import concourse.bass as bass
    import concourse.tile as tile
    from concourse import bacc, mybir

    f32 = mybir.dt.float32
    f32r = mybir.dt.float32r
    bf16 = mybir.dt.bfloat16
    AF = mybir.ActivationFunctionType
    OP = mybir.AluOpType
    AX = mybir.AxisListType

    assert nrows % 16 == 0
    SLAB = nrows + 2
    NCHUNK = B * (SLAB // 2)          # 512-px chunks over the slab
    NBAND = nrows // 16               # FFN bands of 16 output rows
    seg = {r: (nrows // r) * (W // r) for r in SCALES}   # per batch
    segoff = {}
    off = 0
    for r in SCALES:
        segoff[r] = off
        off += seg[r]
    nqkT = off // 128                 # qkT 128-px chunks per batch
    MH = [C, C, HID - 2 * C]          # FFN hidden blocks: 128,128,84

    nc = bacc.Bacc("TRN2", target_bir_lowering=False, debug=False,
                   num_devices=n_cores)

    def din(name, shape, dt=f32):
        return nc.dram_tensor(name, shape, dt, kind="ExternalInput").ap()

    xs = din("xs", [B, C, SLAB, W], f32r)
    wqk = din("wqk", [NTAPS, C, 2 * CO], f32r)
    bqk_d = din("bqk", [2 * CO, NS])
    wv_d = din("wv", [C, C], f32r)
    bv_d = din("bv", [C, 1])
    wpj_d = din("wpj", [C, C], f32r)
    n1w_d = din("n1w", [C, 1])
    n1b_d = din("n1b", [C, 1])
    n2w_d = din("n2w", [C, 1])
    n2b_d = din("n2b", [C, 1])
    wf_d = din("wf", [9, C, 2 * HID], bf16)
    wpo_d = din("wpo", [C, 3, C], f32r)  # [hid-in (part), hid-block, ch-out]
    tvec = din("tvec", [1, NS])
    eye2 = din("eye2", [2 * CO, CO])
    ones_d = din("ones", [C, C], f32r)
    hmask = din("hmask", [1, 2])
    out_d = nc.dram_tensor("out", [B, C, nrows, W], f32,
                           kind="ExternalOutput").ap()
    if debug:
        dbg_y = nc.dram_tensor("dbg_y", [C, B, SLAB, W], f32,
                               kind="ExternalOutput").ap()
        dbg_acc = nc.dram_tensor("dbg_acc", [2 * CO, NU, 2 * CO + 1], f32,
                                 kind="ExternalOutput").ap()
        dbg_attn = nc.dram_tensor("dbg_attn", [C, B, C], mybir.dt.bfloat16,
                                  kind="ExternalOutput").ap()
        dbg_xmid = nc.dram_tensor("dbg_xmid", [C, B, SLAB, W], f32,
                                  kind="ExternalOutput").ap()

    def r32(ap):
        return ap.bitcast(f32r)

    with tile.TileContext(nc) as tc:
        with tc.tile_pool(name="wpers", bufs=1) as wp, \
             tc.tile_pool(name="xbig", bufs=1) as xp:

            def load(nm, shape, src, dt=f32):
                t = wp.tile(shape, dt, name=nm, tag=nm)
                nc.sync.dma_start(t[:], src)
                return t

            wv_s = load("wv_s", [C, C], wv_d[:], dt=f32r)
            wpj_s = load("wpj_s", [C, C], wpj_d[:], dt=f32r)
            n1w_s = load("n1w_s", [C, 1], n1w_d[:])
            n1b_s = load("n1b_s", [C, 1], n1b_d[:])
            n2w_s = load("n2w_s", [C, 1], n2w_d[:])
            n2b_s = load("n2b_s", [C, 1], n2b_d[:])
            bv_s = load("bv_s", [C, 1], bv_d[:])
            bqk_s = load("bqk_s", [2 * CO, NS], bqk_d[:])
            ones_s = load("ones_s", [C, C], ones_d[:], dt=f32r)
            eye_s = load("eye_s", [2 * CO, CO], eye2[:])
            t_s = load("t_s", [2 * CO, NS],
                       bass.AP(tensor=tvec.tensor, offset=tvec.offset,
                               ap=[[0, 2 * CO], [1, NS]]))
            hm_s = load("hm_s", [C, 2],
                        bass.AP(tensor=hmask.tensor, offset=hmask.offset,
                                ap=[[0, C], [1, 2]]))

            attnT_s = wp.tile([C, B, C], bf16)      # block-diag attn^T
            acc_s = wp.tile([2 * CO, NU, 2 * CO + 1], f32)
            nacc_s = wp.tile([2 * CO, NU, 16], f32)
            nc.vector.memset(nacc_s[:], 0.0)
            eps_s = wp.tile([C, 1], f32)
            nc.vector.memset(eps_s[:], EPS)

            x_s = xp.tile([C, B, SLAB, W], f32r)    # x, later x_mid
            nc.sync.dma_start(x_s[:], xs.rearrange("b c h w -> c b h w"))

            def layernorm_chunk(xc, outc, lp, lps):
                """outc = (xc - mean_c(xc)) * rstd over partitions, via
                all-ones matmuls (stats replicated on all partitions)."""
                sh = [C] + list(xc.shape[1:])
                xcf = xc.bitcast(f32)
                sq = lp.tile(sh, f32r, name="sq", tag="lnA")
                nc.scalar.activation(sq[:], xcf, AF.Square)
                s1 = lps.tile(sh, f32, tag="s1")
                nc.tensor.matmul(s1[:], ones_s[:], xc,
                                 start=True, stop=True)
                s2 = lps.tile(sh, f32, tag="s2")
                nc.tensor.matmul(s2[:], ones_s[:], sq[:],
                                 start=True, stop=True)
                mu2 = lp.tile(sh, f32, name="mu2", tag="lnB")
                nc.scalar.activation(mu2[:], s1[:], AF.Square,
                                     scale=1.0 / C)
                var = lp.tile(sh, f32, name="var", tag="lnC")
                nc.vector.scalar_tensor_tensor(
                    var[:], s2[:], 1.0 / C, mu2[:], OP.mult, OP.subtract)
                sig = lp.tile(sh, f32, name="sig", tag="lnA")
                nc.scalar.activation(sig[:], var[:], AF.Sqrt,
                                     bias=eps_s[:, 0:1])
                rstd = lp.tile(sh, f32, name="rstd", tag="lnB")
                nc.vector.reciprocal_approx_fast(rstd[:], sig[:])
                dmu = lp.tile(sh, f32, name="dmu", tag="lnC")
                nc.vector.scalar_tensor_tensor(
                    dmu[:], s1[:], -1.0 / C, xcf, OP.mult, OP.add)
                nc.vector.tensor_mul(outc, dmu[:], rstd[:])

            # ==========================================================
            # Phases 1 / 1.5 / 2a (yhat alive)
            # ==========================================================
            with tc.tile_pool(name="ybig", bufs=1) as yp:
                yhat = yp.tile([C, B, SLAB, W], f32r)

                with tc.tile_pool(name="ln1", bufs=3) as lp, \
                     tc.tile_pool(name="ln1ps", bufs=2, space="PSUM") as lps:
                    for ch in range(NCHUNK):
                        b, rp = divmod(ch, SLAB // 2)
                        layernorm_chunk(x_s[:, b, 2 * rp:2 * rp + 2, :],
                                        yhat[:, b, 2 * rp:2 * rp + 2, :],
                                        lp, lps)

                # ---- qk convs, transpose, Gram + norms ---------------
                with tc.tile_pool(name="qkw", bufs=1) as qwp, \
                     tc.tile_pool(name="qkTp", bufs=1) as qtp, \
                     tc.tile_pool(name="qkst", bufs=4) as qsp, \
                     tc.tile_pool(name="qkps", bufs=2, space="PSUM") as qps, \
                     tc.tile_pool(name="grps", bufs=1, space="PSUM") as gps, \
                     tc.tile_pool(name="nscp", bufs=3) as nscp:
                    qkT = qtp.tile([C, B, nqkT, 2 * CO], bf16)

                    for si, r in enumerate(SCALES):
                        pr, pc = nrows // r, W // r
                        t0 = sum(s * s for s in SCALES[:si])
                        wqk_s = qwp.tile([C, r * r * 2 * CO], f32r,
                                         name="wqk_s", tag="wqk_s")
                        nc.sync.dma_start(
                            wqk_s[:],
                            bass.AP(tensor=wqk.tensor,
                                    offset=wqk.offset + t0 * C * 2 * CO,
                                    ap=[[2 * CO, C], [C * 2 * CO, r * r],
                                        [1, 2 * CO]]))
                        batched = (pr * pc) < 256
                        ppc = min(max(1, 512 // pc), pr)
                        nck = (pr + ppc - 1) // ppc
                        grams = [gps.tile([2 * CO, 2 * CO], f32,
                                          name=f"gram{b}", tag=f"g{b}")
                                 for b in range(B)]
                        nmm = [0, 0]
                        for ck in range(nck):
                            q0 = ck * ppc
                            rws = min(ppc, pr - q0)
                            npx = rws * pc
                            ps = qps.tile([2 * CO, B, npx] if batched
                                          else [2 * CO, 512], f32, tag="qps")
                            for ti in range(r * r):
                                dy, dx = divmod(ti, r)
                                lhs = wqk_s[:, ti * 2 * CO:
                                            (ti + 1) * 2 * CO]
                                if batched:
                                    rhs = yhat[:, :,
                                               1 + r * q0 + dy:
                                               1 + r * (q0 + rws):r, dx::r]
                                    o = ps[:]
                                else:
                                    rhs = yhat[:, 0,
                                               1 + r * q0 + dy:
                                               1 + r * (q0 + rws):r, dx::r]
                                    o = ps[:, :npx]
                                nc.tensor.matmul(o, lhs, rhs,
                                                 start=(ti == 0),
                                                 stop=(ti == r * r - 1))
                            if not batched:
                                # second batch as its own accumulation
                                ps2 = qps.tile([2 * CO, 512], f32,
                                               tag="qps2")
                                for ti in range(r * r):
                                    dy, dx = divmod(ti, r)
                                    lhs = wqk_s[:, ti * 2 * CO:
                                                (ti + 1) * 2 * CO]
                                    rhs = yhat[:, 1,
                                               1 + r * q0 + dy:
                                               1 + r * (q0 + rws):r, dx::r]
                                    nc.tensor.matmul(ps2[:, :npx], lhs,
                                                     rhs,
                                                     start=(ti == 0),
                                                     stop=(ti == r * r - 1))
                            for b in range(B):
                                u = si * B + b
                                if batched:
                                    psb = ps[:, b, :]
                                elif b == 0:
                                    psb = ps[:, :npx]
                                else:
                                    psb = ps2[:, :npx]
                                st = qsp.tile([2 * CO, 512], bf16, tag="st")
                                nc.scalar.activation(
                                    st[:, :npx], psb, AF.Identity,
                                    bias=bqk_s[:, si:si + 1])
                                nsc = nscp.tile([2 * CO, 512], bf16,
                                                tag="nsc")
                                nc.scalar.activation(
                                    nsc[:, :npx], st[:, :npx], AF.Square,
                                    accum_out=nacc_s[:, u, ck:ck + 1])
                                base = (segoff[r] + q0 * pc) // 128
                                nt = npx // 128
                                nc.sync.dma_start_transpose(
                                    qkT[:, b, base:base + nt, :],
                                    st[:, :npx])
                                for j in range(nt):
                                    nc.tensor.matmul(
                                        grams[b][:], qkT[:, b, base + j, :],
                                        qkT[:, b, base + j, :],
                                        start=(nmm[b] == 0),
                                        stop=(ck == nck - 1 and j == nt - 1),
                                        skip_group_check=True)
                                    nmm[b] += 1
                        for b in range(B):
                            nc.scalar.activation(
                                acc_s[:, si * B + b, 0:2 * CO],
                                grams[b][:], AF.Identity, bias=0.0)
                    nc.vector.reduce_sum(acc_s[:, :, 2 * CO:2 * CO + 1],
                                         nacc_s[:], axis=AX.X)

                if debug:
                    nc.sync.dma_start(dbg_y[:], yhat[:].bitcast(f32))

                # ---- AllReduce of gram+norms -------------------------
                if n_cores > 1:
                    with tc.tile_pool(name="ccd", bufs=1, space="DRAM") as dpp:
                        inb = dpp.tile([2 * CO, NU * (2 * CO + 1)], f32)
                        outb = dpp.tile([2 * CO, NU * (2 * CO + 1)], f32)
                        nc.sync.dma_start(
                            inb[:], acc_s[:].rearrange("p a b -> p (a b)"))
                        nc.gpsimd.collective_compute(
                            "AllReduce", OP.add,
                            replica_groups=[list(range(n_cores))],
                            ins=[inb.opt()], outs=[outb.opt()])
                        nc.sync.dma_start(
                            acc_s[:].rearrange("p a b -> p (a b)"), outb[:])

                # ---- softmax -> block-diag attn^T --------------------
                with tc.tile_pool(name="smx", bufs=1) as sp, \
                     tc.tile_pool(name="smxps", bufs=2, space="PSUM") as sps:
                    nc.vector.memset(attnT_s[:], 0.0)
                    A_st = sp.tile([CO, NU, CO], f32)
                    for u in range(NU):
                        si, b = divmod(u, B)
                        nrm = sp.tile([2 * CO, 1], f32, tag="nrm")
                        nc.scalar.activation(
                            nrm[:], acc_s[:, u, 2 * CO:2 * CO + 1], AF.Sqrt)
                        nc.vector.tensor_scalar(nrm[:], nrm[:], 1e-12,
                                                None, OP.max)
                        rn = sp.tile([2 * CO, 1], f32, tag="rn")
                        nc.vector.reciprocal(rn[:], nrm[:])
                        rq2 = sp.tile([2 * CO, 1], f32, tag="rq2")
                        nc.vector.tensor_mul(rq2[32:64, :], rn[32:64, :],
                                             t_s[32:64, si:si + 1])
                        dq = sp.tile([2 * CO, CO], f32, tag="dq")
                        nc.vector.tensor_scalar(dq[32:64, :],
                                                eye_s[32:64, :],
                                                rq2[32:64, :], None, OP.mult)
                        m1 = sps.tile([CO, CO], f32, tag="m1")
                        nc.tensor.matmul(m1[:], acc_s[32:64, u, 0:CO],
                                         dq[32:64, :], start=True, stop=True)
                        o1 = sp.tile([CO, CO], f32, tag="o1")
                        nc.scalar.activation(o1[:], m1[:], AF.Identity,
                                             bias=0.0)
                        dk = sp.tile([CO, CO], f32, tag="dk")
                        nc.vector.tensor_scalar(dk[:], eye_s[0:32, :],
                                                rn[0:32, :], None, OP.mult)
                        m2 = sps.tile([CO, CO], f32, tag="m2")
                        nc.tensor.matmul(m2[:], o1[:], dk[:],
                                         start=True, stop=True)
                        nc.scalar.activation(A_st[:, u, :], m2[:],
                                             AF.Identity, bias=0.0)
                    negmax = sp.tile([CO, NU], f32)
                    nc.vector.reduce_max(negmax[:], A_st[:], axis=AX.X,
                                         negate=True)
                    E_st = sp.tile([CO, NU, CO], f32)
                    for u in range(NU):
                        nc.scalar.activation(E_st[:, u, :], A_st[:, u, :],
                                             AF.Exp,
                                             bias=negmax[:, u:u + 1])
                    ssum = sp.tile([CO, NU], f32)
                    nc.vector.reduce_sum(ssum[:], E_st[:], axis=AX.X)
                    rs = sp.tile([CO, NU], f32)
                    nc.vector.reciprocal(rs[:], ssum[:])
                    for u in range(NU):
                        si, b = divmod(u, B)
                        at = sp.tile([CO, CO], f32, tag="at")
                        nc.vector.tensor_scalar(at[:], E_st[:, u, :],
                                                rs[:, u:u + 1], None,
                                                OP.mult)
                        att = sp.tile([CO, CO], f32, tag="att")
                        nc.vector.transpose(att[:], at[:])
                        attb = sp.tile([CO, CO], bf16, tag="attb")
                        nc.scalar.activation(attb[:], att[:], AF.Identity,
                                             bias=0.0)
                        nc.sync.dma_start(
                            attnT_s[32 * si:32 * (si + 1), b,
                                    32 * si:32 * (si + 1)], attb[:])

                if debug:
                    nc.sync.dma_start(dbg_acc[:], acc_s[:])
                    nc.sync.dma_start(dbg_attn[:].bitcast(mybir.dt.bfloat16), attnT_s[:])

                # ---- phase 2a: v, attn@v, wproj, x_mid ---------------
                with tc.tile_pool(name="p2a", bufs=4) as ap_, \
                     tc.tile_pool(name="p2aps", bufs=2, space="PSUM") as aps:
                    for ch in range(NCHUNK):
                        b, rp = divmod(ch, SLAB // 2)
                        yc = yhat[:, b, 2 * rp:2 * rp + 2, :]
                        xc = x_s[:, b, 2 * rp:2 * rp + 2, :]
                        vps = aps.tile([C, 2, W], f32, tag="vps")
                        nc.tensor.matmul(vps[:], wv_s[:], yc,
                                         start=True, stop=True)
                        v_sb = ap_.tile([C, 2, W], bf16, tag="v_sb")
                        nc.scalar.activation(v_sb[:], vps[:], AF.Identity,
                                             bias=bv_s[:, 0:1])
                        avps = aps.tile([C, 2, W], f32, tag="avps")
                        nc.tensor.matmul(avps[:], attnT_s[:, b, :],
                                         v_sb[:], start=True, stop=True)
                        av_sb = ap_.tile([C, 2, W], f32r, tag="av_sb")
                        nc.scalar.activation(av_sb[:], avps[:],
                                             AF.Identity, bias=0.0)
                        pjps = aps.tile([C, 2, W], f32, tag="pjps")
                        nc.tensor.matmul(pjps[:], wpj_s[:],
                                         av_sb[:], start=True,
                                         stop=True)
                        t_sb = ap_.tile([C, 2, W], f32, tag="t_sb")
                        nc.vector.scalar_tensor_tensor(
                            t_sb[:], yc.bitcast(f32), n1w_s[:, 0:1], pjps[:],
                            OP.mult, OP.add)
                        nc.vector.tensor_add(xc, xc.bitcast(f32), t_sb[:])
                        nc.gpsimd.tensor_scalar(xc, xc.bitcast(f32),
                                                n1b_s[:, 0:1],
                                                None, OP.add)
            # yhat freed here
            if debug:
                nc.sync.dma_start(dbg_xmid[:], x_s[:].bitcast(f32))

            # ==========================================================
            # Phase 2b: LN2 + folded FFN, band by band
            # ==========================================================
            with tc.tile_pool(name="ffnw", bufs=1) as fwp, \
                 tc.tile_pool(name="zp", bufs=1) as zp, \
                 tc.tile_pool(name="ln2", bufs=3) as lp2, \
                 tc.tile_pool(name="ln2ps", bufs=1, space="PSUM") as lps2, \
                 tc.tile_pool(name="fps", bufs=2, space="PSUM") as fps, \
                 tc.tile_pool(name="ops", bufs=2, space="PSUM") as ops_, \
                 tc.tile_pool(name="gp", bufs=2) as gp, \
                 tc.tile_pool(name="outp", bufs=3) as outp:
                wf_s = fwp.tile([C, 9 * 2 * HID], bf16)
                nc.sync.dma_start(
                    wf_s[:],
                    bass.AP(tensor=wf_d.tensor, offset=wf_d.offset,
                            ap=[[2 * HID, C], [C * 2 * HID, 9],
                                [1, 2 * HID]]))
                wpo_s = fwp.tile([C, 3, C], f32r)
                nc.sync.dma_start(wpo_s[:], wpo_d[:])

                for bd in range(NBAND):
                    zr0 = 16 * bd           # slab row of z-band row 0
                    zt = zp.tile([C, B, 18, W + 2], bf16, tag="zt")
                    # LN2 into padded z band
                    for ch in range(2 * 9):
                        b, rp = divmod(ch, 9)
                        layernorm_chunk(
                            x_s[:, b, zr0 + 2 * rp:zr0 + 2 * rp + 2, :],
                            zt[:, b, 2 * rp:2 * rp + 2, 1:W + 1],
                            lp2, lps2)
                    # scale/shift by n2w/n2b (in place)
                    nc.scalar.activation(zt[:, :, :, 1:W + 1],
                                         zt[:, :, :, 1:W + 1], AF.Identity,
                                         scale=n2w_s[:, 0:1],
                                         bias=n2b_s[:, 0:1])
                    # zero the padding columns
                    nc.vector.memset(zt[:, :, :, 0:1], 0.0)
                    nc.vector.memset(zt[:, :, :, W + 1:W + 2], 0.0)
                    # zero halo rows at image boundary
                    if bd == 0:
                        nc.vector.tensor_scalar(
                            zt[:, :, 0, 1:W + 1], zt[:, :, 0, 1:W + 1],
                            hm_s[:, 0:1], None, OP.mult)
                    if bd == NBAND - 1:
                        nc.vector.tensor_scalar(
                            zt[:, :, 17, 1:W + 1], zt[:, :, 17, 1:W + 1],
                            hm_s[:, 1:2], None, OP.mult)

                    for ch in range(16):
                        b, rp = divmod(ch, 8)
                        ops = ops_.tile([C, 2, W], f32, tag="ops")
                        for p in range(3):
                            mh = MH[p]
                            f1 = fps.tile([C, 2, W], f32, tag="f1")
                            f2 = fps.tile([C, 2, W], f32, tag="f2")
                            for ti in range(9):
                                dy, dx = divmod(ti, 3)
                                rhs = zt[:, b, 2 * rp + dy:2 * rp + dy + 2,
                                         dx:W + dx]
                                c1 = 128 * p
                                nc.tensor.matmul(
                                    f1[:mh], wf_s[:, 2 * HID * ti + c1:
                                                  2 * HID * ti + c1 + mh],
                                    rhs, start=(ti == 0), stop=(ti == 8))
                                c2 = HID + 128 * p
                                nc.tensor.matmul(
                                    f2[:mh], wf_s[:, 2 * HID * ti + c2:
                                                  2 * HID * ti + c2 + mh],
                                    rhs, start=(ti == 0), stop=(ti == 8))
                            g1 = gp.tile([C, 2, W], f32, tag="g1")
                            nc.scalar.activation(g1[:mh], f1[:mh], AF.Gelu)
                            g = gp.tile([C, 2, W], f32r, tag="g")
                            nc.vector.tensor_mul(g[:mh], g1[:mh], f2[:mh])
                            nc.tensor.matmul(ops[:], wpo_s[:mh, p, :],
                                             g[:mh],
                                             start=(p == 0), stop=(p == 2))
                        o_sb = outp.tile([C, 2, W], f32, tag="o_sb")
                        nc.vector.scalar_tensor_tensor(
                            o_sb[:], x_s[:, b, zr0 + 1 + 2 * rp:
                                         zr0 + 3 + 2 * rp, :].bitcast(f32),
                            1.0, ops[:], OP.bypass, OP.add)
                        gr = 16 * bd + 2 * rp
                        nc.sync.dma_start(out_d[b, :, gr:gr + 2, :],
                                          o_sb[:])

    nc.compile()
    return nc


# ---------------------------------------------------------------------------
# host side
# ---------------------------------------------------------------------------

def _prep_inputs(inputs, nrows, n_cores):
    """Precompute folded weights (shared across cores) and per-core
    sliced/padded x slabs."""
    H = nrows * n_cores
    x = np.asarray(inputs["x"], np.float32)
    n1w = np.asarray(inputs["n1w"], np.float32)
    n1b = np.asarray(inputs["n1b"], np.float32)
    n2w = np.asarray(inputs["n2w"], np.float32)
    n2b = np.asarray(inputs["n2b"], np.float32)

    wqk_taps = np.zeros((NTAPS, C, 2 * CO), np.float32)
    bqk = np.zeros((2 * CO, NS), np.float32)
    ti = 0
    for si, r in enumerate(SCALES):
        wqk = np.asarray(inputs[f"wqk{si}"], np.float32)  # [64,128,r,r]
        wqkf = wqk * n1w[None, :, None, None]
        bqk[:, si] = np.einsum("ocyx,c->o", wqk, n1b)
        for dy in range(r):
            for dx in range(r):
                wqk_taps[ti] = wqkf[:, :, dy, dx].T
                ti += 1

    wv_cat = np.concatenate([np.asarray(inputs[f"wv{i}"], np.float32)[:, :, 0, 0]
                             for i in range(NS)], axis=0)      # [128,128]
    bv_cat = np.concatenate([np.asarray(inputs[f"bv{i}"], np.float32)
                             for i in range(NS)])
    bv_all = (wv_cat @ n1b + bv_cat).astype(np.float32)
    wv_t = (wv_cat * n1w[None, :]).T.copy()                    # [in,out]

    wpj_t = np.asarray(inputs["wproj"], np.float32)[:, :, 0, 0].T.copy()

    wpi = np.asarray(inputs["wpi"], np.float32)[:, :, 0, 0]    # [680,128]
    wdw = np.asarray(inputs["wdw"], np.float32)[:, 0]          # [680,3,3]
    import ml_dtypes
    wf = np.zeros((9, C, 2 * HID), np.float32)
    for ti in range(9):
        dy, dx = divmod(ti, 3)
        wf[ti] = (wpi * wdw[:, dy, dx][:, None]).T             # [128,680]
    wf = wf.astype(ml_dtypes.bfloat16)

    wpo = np.asarray(inputs["wpo"], np.float32)[:, :, 0, 0]    # [128,340]
    wpo_p = np.zeros((C, 3, C), np.float32)
    for p in range(3):
        mh = min(C, HID - C * p)
        wpo_p[:mh, p, :] = wpo[:, C * p:C * p + mh].T

    tv = np.array([[float(np.asarray(inputs[f"t{i}"]).reshape(-1)[0])
                    for i in range(NS)]], np.float32)
    eye2 = np.concatenate([np.eye(CO, dtype=np.float32)] * 2, axis=0)

    shared = {
        "wqk": wqk_taps, "bqk": bqk, "wv": wv_t,
        "bv": bv_all.reshape(C, 1), "wpj": wpj_t,
        "n1w": n1w.reshape(C, 1), "n1b": n1b.reshape(C, 1),
        "n2w": n2w.reshape(C, 1), "n2b": n2b.reshape(C, 1),
        "wf": wf, "wpo": wpo_p, "tvec": tv, "eye2": eye2,
        "ones": np.ones((C, C), np.float32),
    }

    in_maps = []
    for i in range(n_cores):
        r0 = nrows * i
        slab = np.zeros((B, C, nrows + 2, W), np.float32)
        lo, hi = r0 - 1, r0 + nrows + 1
        slo, shi = max(lo, 0), min(hi, H)
        slab[:, :, slo - lo:shi - lo, :] = x[:, :, slo:shi, :]
        m = {"xs": slab,
             "hmask": np.array([[1.0 if i > 0 else 0.0,
                                 1.0 if i < n_cores - 1 else 0.0]],
                               np.float32)}
        m.update(shared)
        in_maps.append(m)
    return in_maps


def _run(nrows, n_cores, in_maps, trace=False):
    from concourse.bass_utils import run_bass_kernel_spmd
    key = (nrows, n_cores)
    if key not in _CACHE:
        _CACHE[key] = _build(nrows, n_cores)
    nc = _CACHE[key]
    return run_bass_kernel_spmd(nc, in_maps, core_ids=list(range(n_cores)),
                                trace=trace)


def run_sharded(inputs, nrows=32, n_cores=8, trace=False):
    in_maps = _prep_inputs(inputs, nrows, n_cores)
    res = _run(nrows, n_cores, in_maps, trace=trace)
    H = nrows * n_cores
    out = np.zeros((B, C, H, W), np.float32)
    for i in range(n_cores):
        out[:, :, nrows * i:nrows * (i + 1), :] = res.results[i]["out"]
    return out, res


def kernel(**inputs):
    out, _ = run_sharded(inputs, nrows=32, n_cores=8)
    return out



# revision 8
# speedup vs baseline: 1.3613x; 1.3613x over previous
"""Trainium2 Bass kernel for MultiScaleChannelTransformerBlock.

kernel(**inputs) takes the FULL inputs (as produced by setup_inputs())
and returns the FULL output [2, 128, 256, 256] float32.

Sharding: spatial over H across 8 NeuronCores (32 rows each, plus a
1-row halo on each side, host-padded).  The only cross-core
communication is an AllReduce of the per-(batch,scale) attention q-k
Gram blocks and q/k squared norms (tiny).

Structure (per core):
  A: LN1 chunks with the scale-1 qk conv chunks interleaved (lag 2),
     then scales 2/4/8; qkT via DMA transpose feeds per-unit gram
     accumulation + squared-norm accumulators.
  B: AllReduce of gram+norms overlapped with v = wv@yhat for the
     whole slab.
  C: phase-major batched softmax -> block-diagonal attn^T (bf16).
  D: chunk-pipelined main loop: attn@v + wproj + residuals (2a),
     LN2 into a persistent padded z slab, and the folded-FFN chunk
     (9 PSUM-accumulated matmuls against shifted z views) trailing
     2 chunks behind, so the PE stays dense to the end.

All matmul operands are bf16 (fp32 moving operands stream at half
rate on the PE); accumulation is fp32 in PSUM.
"""

import numpy as np

B = 2
C = 128
CO = 32
HID = 340
W = 256
SCALES = [1, 2, 4, 8]
NS = len(SCALES)
NU = NS * B
NTAPS = sum(r * r for r in SCALES)  # 85
EPS = 1e-5

_CACHE = {}


def _build(nrows, n_cores):
    import concourse.bass as bass
    import concourse.tile as tile
    from concourse import bacc, mybir

    f32 = mybir.dt.float32
    bf16 = mybir.dt.bfloat16
    AF = mybir.ActivationFunctionType
    OP = mybir.AluOpType
    AX = mybir.AxisListType

    assert nrows % 2 == 0
    SLAB = nrows + 2
    NCH = SLAB // 2                   # ln/2a chunks per batch (17)
    NFF = nrows // 2                  # ffn chunks per batch (16)
    seg = {r: (nrows // r) * (W // r) for r in SCALES}   # px per batch
    segoff = {}
    off = 0
    for r in SCALES:
        segoff[r] = off
        off += seg[r]
    nqkT = off // 128                 # qkT 128-px tiles per batch (85)
    MH = [C, C, HID - 2 * C]          # FFN hidden blocks: 128,128,84
    NACC = 16                         # norm accum slots per unit

    nc = bacc.Bacc("TRN2", target_bir_lowering=False, debug=False,
                   num_devices=n_cores)

    def din(name, shape, dt=f32):
        return nc.dram_tensor(name, shape, dt, kind="ExternalInput").ap()

    xs = din("xs", [B, C, SLAB, W], bf16)
    wqk = din("wqk", [NTAPS, C, 2 * CO], bf16)
    bqk_d = din("bqk", [2 * CO, NS])
    wv_d = din("wv", [C, C], bf16)
    bv_d = din("bv", [C, 1])
    wpj_d = din("wpj", [C, C], bf16)
    n1w_d = din("n1w", [C, 1])
    n1b_d = din("n1b", [C, 1])
    n2w_d = din("n2w", [C, 1])
    n2b_d = din("n2b", [C, 1])
    wf_d = din("wf", [9, C, 2 * HID], bf16)
    wpo_d = din("wpo", [C, 3, C], bf16)  # [hid-in(part), hid-block, out]
    tvec = din("tvec", [1, NU])
    eye_d = din("eye", [CO, CO])
    ones_d = din("ones", [C, C], bf16)
    hmask = din("hmask", [1, 2])
    out_d = nc.dram_tensor("out", [B, C, nrows, W], f32,
                           kind="ExternalOutput").ap()

    with tile.TileContext(nc) as tc:
        with tc.tile_pool(name="wpers", bufs=1) as wp, \
             tc.tile_pool(name="xbig", bufs=1) as xp, \
             tc.tile_pool(name="ybig", bufs=1) as yp, \
             tc.tile_pool(name="vbig", bufs=1) as vp, \
             tc.tile_pool(name="zbig", bufs=1) as zp:

            def load(nm, shape, src, dt=f32):
                t = wp.tile(shape, dt, name=nm, tag=nm)
                nc.sync.dma_start(t[:], src)
                return t

            wv_s = load("wv_s", [C, C], wv_d[:], dt=bf16)
            wpj_s = load("wpj_s", [C, C], wpj_d[:], dt=bf16)
            n1w_s = load("n1w_s", [C, 1], n1w_d[:])
            n1b_s = load("n1b_s", [C, 1], n1b_d[:])
            n2w_s = load("n2w_s", [C, 1], n2w_d[:])
            n2b_s = load("n2b_s", [C, 1], n2b_d[:])
            bv_s = load("bv_s", [C, 1], bv_d[:])
            bqk_s = load("bqk_s", [2 * CO, NS], bqk_d[:])
            ones_s = load("ones_s", [C, C], ones_d[:], dt=bf16)
            eye_s = load("eye_s", [CO, CO], eye_d[:])
            # temperature broadcast to 32 partitions (row per unit)
            t_s = load("t_s", [CO, NU],
                       bass.AP(tensor=tvec.tensor, offset=tvec.offset,
                               ap=[[0, CO], [1, NU]]))
            hm_s = load("hm_s", [C, 2],
                        bass.AP(tensor=hmask.tensor, offset=hmask.offset,
                                ap=[[0, C], [1, 2]]))
            wf_s = wp.tile([C, 9 * 2 * HID], bf16, name="wf_s", tag="wf_s")
            nc.sync.dma_start(
                wf_s[:],
                bass.AP(tensor=wf_d.tensor, offset=wf_d.offset,
                        ap=[[2 * HID, C], [C * 2 * HID, 9], [1, 2 * HID]]))
            wpo_s = load("wpo_s", [C, 3, C], wpo_d[:], dt=bf16)

            attnT_s = wp.tile([C, B, C], bf16)      # block-diag attn^T
            # acc: [0:32, u, 0:32] = q-k gram; [:, u, 32] = sq-norms (k|q)
            acc_s = wp.tile([2 * CO, NU, CO + 1], f32)
            nacc_s = wp.tile([2 * CO, NU, NACC], f32)
            nc.vector.memset(nacc_s[:], 0.0)
            nc.vector.memset(attnT_s[:], 0.0)
            eps_s = wp.tile([C, 1], f32)
            nc.vector.memset(eps_s[:], EPS)

            x_s = xp.tile([C, B, SLAB, W], bf16)    # x, later x_mid
            nc.sync.dma_start(x_s[:], xs.rearrange("b c h w -> c b h w"))
            yhat = yp.tile([C, B, SLAB, W], bf16)
            v_sb = vp.tile([C, B, SLAB, W], bf16)
            zt = zp.tile([C, B, SLAB, W + 2], bf16)

            def ln_chunk(xc, outc, lp, lps, affine=False):
                """outc = (xc - mean_c(xc)) * rstd over partitions,
                optionally * n2w + n2b.  Stats via all-ones matmuls."""
                sh = [C] + list(xc.shape[1:])
                sq = lp.tile(sh, bf16, name="sq", tag="sq")
                nc.scalar.activation(sq[:], xc, AF.Square)
                s1 = lps.tile(sh, f32, tag="s1")
                nc.tensor.matmul(s1[:], ones_s[:], xc, start=True, stop=True)
                s2 = lps.tile(sh, f32, tag="s2")
                nc.tensor.matmul(s2[:], ones_s[:], sq[:],
                                 start=True, stop=True)
                mu2 = lp.tile(sh, f32, name="mu2", tag="mu2")
                nc.scalar.activation(mu2[:], s1[:], AF.Square, scale=1.0 / C)
                var = lp.tile(sh, f32, name="var", tag="var")
                nc.vector.scalar_tensor_tensor(
                    var[:], s2[:], 1.0 / C, mu2[:], OP.mult, OP.subtract)
                sig = lp.tile(sh, f32, name="sig", tag="sig")
                nc.scalar.activation(sig[:], var[:], AF.Sqrt,
                                     bias=eps_s[:, 0:1])
                rstd = lp.tile(sh, f32, name="rstd", tag="rstd")
                nc.vector.reciprocal_approx_fast(rstd[:], sig[:])
                dmu = lp.tile(sh, bf16, name="dmu", tag="dmu")
                nc.vector.scalar_tensor_tensor(
                    dmu[:], s1[:], -1.0 / C, xc, OP.mult, OP.add)
                nc.vector.tensor_mul(outc, dmu[:], rstd[:])
                if affine:
                    nc.scalar.activation(outc, outc, AF.Identity,
                                         scale=n2w_s[:, 0:1],
                                         bias=n2b_s[:, 0:1])

            # ==========================================================
            # Phase A: LN1 + qk convs, qkT transpose, gram + norms
            # ==========================================================
            NMM_U = {0: 64, 1: 16, 2: 4, 3: 1}  # gram matmuls per unit
            with tc.tile_pool(name="ln1", bufs=2) as lp, \
                 tc.tile_pool(name="ln1ps", bufs=2, space="PSUM") as lps, \
                 tc.tile_pool(name="qkw", bufs=1) as qwp, \
                 tc.tile_pool(name="qkst", bufs=4) as qsp, \
                 tc.tile_pool(name="qkps", bufs=2, space="PSUM") as qps, \
                 tc.tile_pool(name="grps", bufs=1, space="PSUM") as gps:
                wqk0_s = qwp.tile([C, 2 * CO], bf16, name="wqk0_s",
                                  tag="wqk0")
                nc.sync.dma_start(wqk0_s[:], wqk[0])
                gram_ps = gps.tile([CO, NU, CO], f32)
                nmm = [0] * NU

                def qk_chunk(b, si, r, ck, wqk_sl):
                    pr, pc = nrows // r, W // r
                    ppc = min(max(1, 512 // pc), pr)
                    q0 = ck * ppc
                    rws = min(ppc, pr - q0)
                    npx = rws * pc
                    u = si * B + b
                    ps = qps.tile([2 * CO, 512], f32, tag="qps")
                    for ti in range(r * r):
                        dy, dx = divmod(ti, r)
                        nc.tensor.matmul(
                            ps[:, :npx],
                            wqk_sl[:, ti * 2 * CO:(ti + 1) * 2 * CO],
                            yhat[:, b, 1 + r * q0 + dy:1 + r * (q0 + rws):r,
                                 dx::r],
                            start=(ti == 0), stop=(ti == r * r - 1))
                    st = qsp.tile([2 * CO, 512], bf16, tag="st")
                    nc.scalar.activation(st[:, :npx], ps[:, :npx],
                                         AF.Identity,
                                         bias=bqk_s[:, si:si + 1])
                    nsc = qsp.tile([2 * CO, 512], bf16, tag="nsc")
                    nc.scalar.activation(nsc[:, :npx], st[:, :npx],
                                         AF.Square,
                                         accum_out=nacc_s[:, u, ck:ck + 1])
                    nt = npx // 128
                    qkt = qsp.tile([C, 4, 2 * CO], bf16, tag="qkt")
                    nc.sync.dma_start_transpose(
                        qkt[:, 0:nt, :], st[:, :npx])
                    for j in range(nt):
                        nc.tensor.matmul(
                            gram_ps[:, u, :], qkt[:, j, CO:2 * CO],
                            qkt[:, j, 0:CO],
                            start=(nmm[u] == 0),
                            stop=(nmm[u] == NMM_U[si] - 1),
                            skip_group_check=True)
                        nmm[u] += 1

                for b in range(B):
                    for m in range(NCH + 1):
                        if m < NCH:
                            ln_chunk(x_s[:, b, 2 * m:2 * m + 2, :],
                                     yhat[:, b, 2 * m:2 * m + 2, :],
                                     lp, lps)
                        if m >= 2:
                            qk_chunk(b, 0, 1, m - 2, wqk0_s[:])
                    for si, r in enumerate(SCALES):
                        if si == 0:
                            continue
                        t0 = sum(s * s for s in SCALES[:si])
                        wqk_s = qwp.tile([C, r * r * 2 * CO], bf16,
                                         name="wqk_s", tag="wqk_s")
                        nc.sync.dma_start(
                            wqk_s[:],
                            bass.AP(tensor=wqk.tensor,
                                    offset=wqk.offset + t0 * C * 2 * CO,
                                    ap=[[2 * CO, C], [C * 2 * CO, r * r],
                                        [1, 2 * CO]]))
                        pr, pc = nrows // r, W // r
                        ppc = min(max(1, 512 // pc), pr)
                        nck = (pr + ppc - 1) // ppc
                        for ck in range(nck):
                            qk_chunk(b, si, r, ck, wqk_s[:])
                nc.scalar.activation(acc_s[0:CO, :, 0:CO], gram_ps[:],
                                     AF.Identity, bias=0.0)
                nc.vector.reduce_sum(acc_s[:, :, CO:CO + 1], nacc_s[:],
                                     axis=AX.X)

            # ==========================================================
            # Phase B: AllReduce of gram+norms || v = wv @ yhat
            # ==========================================================
            with tc.tile_pool(name="vps", bufs=2, space="PSUM") as vpp:
                if n_cores > 1:
                    with tc.tile_pool(name="ccd", bufs=1,
                                      space="DRAM") as dpp:
                        inb = dpp.tile([2 * CO, NU * (CO + 1)], f32)
                        outb = dpp.tile([2 * CO, NU * (CO + 1)], f32)
                        nc.sync.dma_start(
                            inb[:], acc_s[:].rearrange("p a b -> p (a b)"))
                        nc.gpsimd.collective_compute(
                            "AllReduce", OP.add,
                            replica_groups=[list(range(n_cores))],
                            ins=[inb.opt()], outs=[outb.opt()])
                        for b in range(B):
                            for m in range(NCH):
                                ps = vpp.tile([C, 2, W], f32, tag="vps")
                                nc.tensor.matmul(
                                    ps[:], wv_s[:],
                                    yhat[:, b, 2 * m:2 * m + 2, :],
                                    start=True, stop=True)
                                nc.scalar.activation(
                                    v_sb[:, b, 2 * m:2 * m + 2, :], ps[:],
                                    AF.Identity, bias=bv_s[:, 0:1])
                        nc.sync.dma_start(
                            acc_s[:].rearrange("p a b -> p (a b)"), outb[:])
                else:
                    for b in range(B):
                        for m in range(NCH):
                            ps = vpp.tile([C, 2, W], f32, tag="vps")
                            nc.tensor.matmul(
                                ps[:], wv_s[:],
                                yhat[:, b, 2 * m:2 * m + 2, :],
                                start=True, stop=True)
                            nc.scalar.activation(
                                v_sb[:, b, 2 * m:2 * m + 2, :], ps[:],
                                AF.Identity, bias=bv_s[:, 0:1])

            # ==========================================================
            # Phase C: softmax -> block-diag attn^T (phase-major)
            # ==========================================================
            with tc.tile_pool(name="smx", bufs=1) as sp, \
                 tc.tile_pool(name="smxps", bufs=2, space="PSUM") as sps:
                # q norms down to partitions 0:32 next to k norms
                nrm2 = sp.tile([CO, 2, NU], f32)   # [:,0,:]=k  [:,1,:]=q
                nc.scalar.activation(
                    nrm2[:, 0, :],
                    acc_s[0:CO, :, CO:CO + 1].rearrange("p a b -> p (a b)"),
                    AF.Sqrt)
                qn = sp.tile([CO, NU], f32)
                nc.sync.dma_start(
                    qn[:],
                    acc_s[CO:2 * CO, :, CO:CO + 1].rearrange(
                        "p a b -> p (a b)"))
                nc.scalar.activation(nrm2[:, 1, :], qn[:], AF.Sqrt)
                nc.vector.tensor_scalar(nrm2[:], nrm2[:], 1e-12, None,
                                        OP.max)
                rn2 = sp.tile([CO, 2, NU], f32)
                nc.vector.reciprocal(rn2[:], nrm2[:])
                # q side gets the temperature folded in
                nc.vector.tensor_mul(rn2[:, 1, :], rn2[:, 1, :], t_s[:])
                dq = sp.tile([CO, NU, CO], f32)
                dk = sp.tile([CO, NU, CO], f32)
                for u in range(NU):
                    nc.vector.tensor_scalar(dq[:, u, :], eye_s[:],
                                            rn2[:, 1, u:u + 1], None,
                                            OP.mult)
                for u in range(NU):
                    nc.vector.tensor_scalar(dk[:, u, :], eye_s[:],
                                            rn2[:, 0, u:u + 1], None,
                                            OP.mult)
                o1 = sp.tile([CO, NU, CO], f32)
                for u in range(NU):
                    m1 = sps.tile([CO, CO], f32, tag="m1")
                    nc.tensor.matmul(m1[:], acc_s[0:CO, u, 0:CO],
                                     dq[:, u, :], start=True, stop=True)
                    nc.scalar.activation(o1[:, u, :], m1[:], AF.Identity,
                                         bias=0.0)
                A_st = sp.tile([CO, NU, CO], f32)
                for u in range(NU):
                    m2 = sps.tile([CO, CO], f32, tag="m2")
                    nc.tensor.matmul(m2[:], o1[:, u, :], dk[:, u, :],
                                     start=True, stop=True)
                    nc.scalar.activation(A_st[:, u, :], m2[:], AF.Identity,
                                         bias=0.0)
                negmax = sp.tile([CO, NU], f32)
                nc.vector.reduce_max(negmax[:], A_st[:], axis=AX.X,
                                     negate=True)
                E_st = sp.tile([CO, NU, CO], f32)
                for u in range(NU):
                    nc.scalar.activation(E_st[:, u, :], A_st[:, u, :],
                                         AF.Exp, bias=negmax[:, u:u + 1])
                ssum = sp.tile([CO, NU], f32)
                nc.vector.reduce_sum(ssum[:], E_st[:], axis=AX.X)
                rs = sp.tile([CO, NU], f32)
                nc.vector.reciprocal(rs[:], ssum[:])
                at = sp.tile([CO, NU, CO], f32)
                for u in range(NU):
                    nc.vector.tensor_scalar(at[:, u, :], E_st[:, u, :],
                                            rs[:, u:u + 1], None, OP.mult)
                att = sp.tile([CO, NU, CO], f32)
                for u in range(NU):
                    nc.vector.transpose(att[:, u, :], at[:, u, :])
                attb = sp.tile([CO, NU, CO], bf16)
                for u in range(NU):
                    nc.scalar.activation(attb[:, u, :], att[:, u, :],
                                         AF.Identity, bias=0.0)
                for u in range(NU):
                    si, b = divmod(u, B)
                    nc.sync.dma_start(
                        attnT_s[CO * si:CO * (si + 1), b,
                                CO * si:CO * (si + 1)], attb[:, u, :])

            # ==========================================================
            # Phase D: 2a + LN2 + folded FFN, chunk-pipelined (lag 2)
            # ==========================================================
            with tc.tile_pool(name="p2a", bufs=3) as ap_, \
                 tc.tile_pool(name="p2aps", bufs=1, space="PSUM") as aps, \
                 tc.tile_pool(name="ln2", bufs=2) as lp2, \
                 tc.tile_pool(name="ln2ps", bufs=1, space="PSUM") as lps2, \
                 tc.tile_pool(name="fps", bufs=2, space="PSUM") as fps, \
                 tc.tile_pool(name="ops", bufs=1, space="PSUM") as ops_, \
                 tc.tile_pool(name="gp", bufs=2) as gp, \
                 tc.tile_pool(name="outp", bufs=3) as outp:
                # zero the z padding columns once
                nc.vector.memset(zt[:, :, :, 0:1], 0.0)
                nc.vector.memset(zt[:, :, :, W + 1:W + 2], 0.0)

                def chunk_2a(b, k):
                    yc = yhat[:, b, 2 * k:2 * k + 2, :]
                    xc = x_s[:, b, 2 * k:2 * k + 2, :]
                    vc = v_sb[:, b, 2 * k:2 * k + 2, :]
                    avps = aps.tile([C, 2, W], f32, tag="a2")
                    nc.tensor.matmul(avps[:], attnT_s[:, b, :], vc,
                                     start=True, stop=True)
                    av = ap_.tile([C, 2, W], bf16, tag="av")
                    nc.scalar.activation(av[:], avps[:], AF.Identity,
                                         bias=0.0)
                    pjps = aps.tile([C, 2, W], f32, tag="a2")
                    nc.tensor.matmul(pjps[:], wpj_s[:], av[:],
                                     start=True, stop=True)
                    tA = ap_.tile([C, 2, W], f32, tag="tA")
                    nc.vector.scalar_tensor_tensor(
                        tA[:], pjps[:], n1b_s[:, 0:1], xc, OP.add, OP.add)
                    nc.vector.scalar_tensor_tensor(
                        xc, yc, n1w_s[:, 0:1], tA[:], OP.mult, OP.add)

                def chunk_ln2(b, k):
                    ln_chunk(x_s[:, b, 2 * k:2 * k + 2, :],
                             zt[:, b, 2 * k:2 * k + 2, 1:W + 1],
                             lp2, lps2, affine=True)
                    if k == 0:
                        nc.vector.tensor_scalar(
                            zt[:, b, 0, 1:W + 1], zt[:, b, 0, 1:W + 1],
                            hm_s[:, 0:1], None, OP.mult)
                    if k == NCH - 1:
                        nc.vector.tensor_scalar(
                            zt[:, b, SLAB - 1, 1:W + 1],
                            zt[:, b, SLAB - 1, 1:W + 1],
                            hm_s[:, 1:2], None, OP.mult)

                def chunk_ffn(b, j):
                    ops = ops_.tile([C, 2, W], f32, tag="ops")
                    for p in range(3):
                        mh = MH[p]
                        f1 = fps.tile([C, 2, W], f32, tag="f1")
                        f2 = fps.tile([C, 2, W], f32, tag="f2")
                        for ti in range(9):
                            dy, dx = divmod(ti, 3)
                            rhs = zt[:, b, 2 * j + dy:2 * j + dy + 2,
                                     dx:W + dx]
                            c1 = 128 * p
                            nc.tensor.matmul(
                                f1[:mh], wf_s[:, 2 * HID * ti + c1:
                                              2 * HID * ti + c1 + mh],
                                rhs, start=(ti == 0), stop=(ti == 8))
                            c2 = HID + 128 * p
                            nc.tensor.matmul(
                                f2[:mh], wf_s[:, 2 * HID * ti + c2:
                                              2 * HID * ti + c2 + mh],
                                rhs, start=(ti == 0), stop=(ti == 8))
                        g1 = gp.tile([C, 2, W], bf16, tag="g1")
                        nc.scalar.activation(g1[:mh], f1[:mh], AF.Gelu)
                        g = gp.tile([C, 2, W], bf16, tag="g")
                        nc.vector.tensor_mul(g[:mh], g1[:mh], f2[:mh])
                        nc.tensor.matmul(ops[:], wpo_s[:mh, p, :], g[:mh],
                                         start=(p == 0), stop=(p == 2))
                    o_sb = outp.tile([C, 2, W], f32, tag="o_sb")
                    nc.vector.tensor_add(
                        o_sb[:], x_s[:, b, 2 * j + 1:2 * j + 3, :], ops[:])
                    nc.sync.dma_start(out_d[b, :, 2 * j:2 * j + 2, :],
                                      o_sb[:])

                for b in range(B):
                    for k in range(NCH + 1):
                        if k < NCH:
                            chunk_2a(b, k)
                            chunk_ln2(b, k)
                        if k >= 2 and k - 2 < NFF:
                            chunk_ffn(b, k - 2)

    nc.compile()
    return nc


# ---------------------------------------------------------------------------
# host side
# ---------------------------------------------------------------------------

def _prep_inputs(inputs, nrows, n_cores):
    """Precompute folded weights (shared across cores) and per-core
    sliced/padded x slabs."""
    import ml_dtypes
    bf = ml_dtypes.bfloat16
    H = nrows * n_cores
    x = np.asarray(inputs["x"], np.float32)
    n1w = np.asarray(inputs["n1w"], np.float32)
    n1b = np.asarray(inputs["n1b"], np.float32)
    n2w = np.asarray(inputs["n2w"], np.float32)
    n2b = np.asarray(inputs["n2b"], np.float32)

    wqk_taps = np.zeros((NTAPS, C, 2 * CO), np.float32)
    bqk = np.zeros((2 * CO, NS), np.float32)
    ti = 0
    for si, r in enumerate(SCALES):
        wqkw = np.asarray(inputs[f"wqk{si}"], np.float32)  # [64,128,r,r]
        wqkf = wqkw * n1w[None, :, None, None]
        bqk[:, si] = np.einsum("ocyx,c->o", wqkw, n1b)
        for dy in range(r):
            for dx in range(r):
                wqk_taps[ti] = wqkf[:, :, dy, dx].T
                ti += 1

    wv_cat = np.concatenate([np.asarray(inputs[f"wv{i}"],
                                        np.float32)[:, :, 0, 0]
                             for i in range(NS)], axis=0)      # [128,128]
    bv_cat = np.concatenate([np.asarray(inputs[f"bv{i}"], np.float32)
                             for i in range(NS)])
    bv_all = (wv_cat @ n1b + bv_cat).astype(np.float32)
    wv_t = (wv_cat * n1w[None, :]).T.copy()                    # [in,out]

    wpj_t = np.asarray(inputs["wproj"], np.float32)[:, :, 0, 0].T.copy()

    wpi = np.asarray(inputs["wpi"], np.float32)[:, :, 0, 0]    # [680,128]
    wdw = np.asarray(inputs["wdw"], np.float32)[:, 0]          # [680,3,3]
    wf = np.zeros((9, C, 2 * HID), np.float32)
    for ti in range(9):
        dy, dx = divmod(ti, 3)
        wf[ti] = (wpi * wdw[:, dy, dx][:, None]).T             # [128,680]

    wpo = np.asarray(inputs["wpo"], np.float32)[:, :, 0, 0]    # [128,340]
    wpo_p = np.zeros((C, 3, C), np.float32)
    for p in range(3):
        mh = min(C, HID - C * p)
        wpo_p[:mh, p, :] = wpo[:, C * p:C * p + mh].T

    tv = np.zeros((1, NU), np.float32)
    for si in range(NS):
        for b in range(B):
            tv[0, si * B + b] = float(
                np.asarray(inputs[f"t{si}"]).reshape(-1)[0])
    eye = np.eye(CO, dtype=np.float32)

    shared = {
        "wqk": wqk_taps.astype(bf), "bqk": bqk,
        "wv": wv_t.astype(bf), "bv": bv_all.reshape(C, 1),
        "wpj": wpj_t.astype(bf),
        "n1w": n1w.reshape(C, 1), "n1b": n1b.reshape(C, 1),
        "n2w": n2w.reshape(C, 1), "n2b": n2b.reshape(C, 1),
        "wf": wf.astype(bf), "wpo": wpo_p.astype(bf),
        "tvec": tv, "eye": eye,
        "ones": np.ones((C, C), np.float32).astype(bf),
    }

    in_maps = []
    for i in range(n_cores):
        r0 = nrows * i
        slab = np.zeros((B, C, nrows + 2, W), np.float32)
        lo, hi = r0 - 1, r0 + nrows + 1
        slo, shi = max(lo, 0), min(hi, H)
        slab[:, :, slo - lo:shi - lo, :] = x[:, :, slo:shi, :]
        m = {"xs": slab.astype(bf),
             "hmask": np.array([[1.0 if i > 0 else 0.0,
                                 1.0 if i < n_cores - 1 else 0.0]],
                               np.float32)}
        m.update(shared)
        in_maps.append(m)
    return in_maps


def _run(nrows, n_cores, in_maps, trace=False):
    from concourse.bass_utils import run_bass_kernel_spmd
    key = (nrows, n_cores)
    if key not in _CACHE:
        _CACHE[key] = _build(nrows, n_cores)
    nc = _CACHE[key]
    return run_bass_kernel_spmd(nc, in_maps, core_ids=list(range(n_cores)),
                                trace=trace)


def run_sharded(inputs, nrows=32, n_cores=8, trace=False):
    in_maps = _prep_inputs(inputs, nrows, n_cores)
    res = _run(nrows, n_cores, in_maps, trace=trace)
    H = nrows * n_cores
    out = np.zeros((B, C, H, W), np.float32)
    for i in range(n_cores):
        out[:, :, nrows * i:nrows * (i + 1), :] = res.results[i]["out"]
    return out, res


def kernel(**inputs):
    out, _ = run_sharded(inputs, nrows=32, n_cores=8)
    return out


# revision 12
# speedup vs baseline: 1.4215x; 1.0442x over previous
"""Trainium2 Bass kernel for MultiScaleChannelTransformerBlock.

kernel(**inputs) takes the FULL inputs (as produced by setup_inputs())
and returns the FULL output [2, 128, 256, 256] float32.

Sharding: spatial over H across 8 NeuronCores (32 rows each, plus a
1-row halo on each side, host-padded).  Cross-core communication is
one small AllReduce per batch image (attention q-k Gram block + q/k
squared norms).

Pipeline (per core), designed so the PE never idles after softmax:
  A:  LN1(b0) with the scale-1 qk conv interleaved, then scales 2/4/8
      -> gram(b0)+norms(b0); AllReduce#0 issued, hidden under a
      prelude of LN1(b1) chunks.
  sm0/K0: batched softmax for b0; wv and bv are folded through the
      attention on-device: K = (attn@Wv)^T, bv_att = attn@bv, so the
      v conv disappears entirely.
  MEGA: the b0 main loop (attn-apply + wproj + residuals, LN2 into a
      persistent padded z slab, folded-FFN trailing 2 chunks behind)
      with the REMAINING b1 prep work (LN1 tail, all qk convs, gram,
      AllReduce#1) slotted into its DVE/ACT slack.  The PE is dense
      on FFN matmuls throughout.
  sm1/K1, then the b1 main loop.

All matmul operands are bf16 (fp32 moving operands stream at half
rate on the PE); accumulation is fp32 in PSUM.
"""

import numpy as np

B = 2
C = 128
CO = 32
HID = 340
W = 256
SCALES = [1, 2, 4, 8]
NS = len(SCALES)
NU = NS * B
NTAPS = sum(r * r for r in SCALES)  # 85
EPS = 1e-5

_CACHE = {}


def _build(nrows, n_cores):
    import concourse.bass as bass
    import concourse.tile as tile
    from concourse import bacc, mybir

    f32 = mybir.dt.float32
    bf16 = mybir.dt.bfloat16
    AF = mybir.ActivationFunctionType
    OP = mybir.AluOpType
    AX = mybir.AxisListType

    assert nrows % 2 == 0
    SLAB = nrows + 2
    NCH = SLAB // 2                   # ln/2a chunks per batch (17)
    NFF = nrows // 2                  # ffn chunks per batch (16)
    seg = {r: (nrows // r) * (W // r) for r in SCALES}
    MH = [C, C, HID - 2 * C]          # FFN hidden blocks: 128,128,84
    NACC = 16                         # norm accum slots per unit
    NMM_U = {0: 64, 1: 16, 2: 4, 3: 1}  # gram matmuls per (scale)
    PRE = 12                          # b1 LN1 chunks issued under AR#0

    nc = bacc.Bacc("TRN2", target_bir_lowering=False, debug=False,
                   num_devices=n_cores)

    def din(name, shape, dt=f32):
        return nc.dram_tensor(name, shape, dt, kind="ExternalInput").ap()

    xs = din("xs", [B, C, SLAB, W], bf16)
    wqk = din("wqk", [NTAPS, C, 2 * CO], bf16)
    bqk_d = din("bqk", [2 * CO, NS])
    wvn_d = din("wvn", [C, C], bf16)     # Wv*n1w, [v_ch, in]
    bv_d = din("bv", [C, 1], bf16)       # Wv@n1b + bv, [v_ch, 1]
    wpj_d = din("wpj", [C, C], bf16)
    n1w_d = din("n1w", [C, 1])
    n1b_d = din("n1b", [C, 1])
    n2w_d = din("n2w", [C, 1])
    n2b_d = din("n2b", [C, 1])
    wf_d = din("wf", [9, C, 2 * HID], bf16)
    wpo_d = din("wpo", [C, 3, C], bf16)
    tvec = din("tvec", [1, NU])          # u = b*NS + si
    eye_d = din("eye", [CO, CO])
    ones_d = din("ones", [C, C], bf16)
    hmask = din("hmask", [1, 2])
    out_d = nc.dram_tensor("out", [B, C, nrows, W], f32,
                           kind="ExternalOutput").ap()

    with tile.TileContext(nc) as tc:
        with tc.tile_pool(name="wpers", bufs=1) as wp, \
             tc.tile_pool(name="xbig", bufs=1) as xp, \
             tc.tile_pool(name="ybig", bufs=1) as yp, \
             tc.tile_pool(name="zbig", bufs=1) as zp, \
             tc.tile_pool(name="ccd", bufs=1, space="DRAM") as dpp:

            def load(nm, shape, src, dt=f32):
                t = wp.tile(shape, dt, name=nm, tag=nm)
                nc.sync.dma_start(t[:], src)
                return t

            wvn_s = load("wvn_s", [C, C], wvn_d[:], dt=bf16)
            wpj_s = load("wpj_s", [C, C], wpj_d[:], dt=bf16)
            n1w_s = load("n1w_s", [C, 1], n1w_d[:])
            n1b_s = load("n1b_s", [C, 1], n1b_d[:])
            n2w_s = load("n2w_s", [C, 1], n2w_d[:])
            n2b_s = load("n2b_s", [C, 1], n2b_d[:])
            bv_s = load("bv_s", [C, 1], bv_d[:], dt=bf16)
            bqk_s = load("bqk_s", [2 * CO, NS], bqk_d[:])
            ones_s = load("ones_s", [C, C], ones_d[:], dt=bf16)
            eye_s = load("eye_s", [CO, CO], eye_d[:])
            t_s = load("t_s", [CO, NU],
                       bass.AP(tensor=tvec.tensor, offset=tvec.offset,
                               ap=[[0, CO], [1, NU]]))
            hm_s = load("hm_s", [C, 2],
                        bass.AP(tensor=hmask.tensor, offset=hmask.offset,
                                ap=[[0, C], [1, 2]]))
            wf_s = wp.tile([C, 9 * 2 * HID], bf16, name="wf_s", tag="wf_s")
            nc.sync.dma_start(
                wf_s[:],
                bass.AP(tensor=wf_d.tensor, offset=wf_d.offset,
                        ap=[[2 * HID, C], [C * 2 * HID, 9], [1, 2 * HID]]))
            wpo_s = load("wpo_s", [C, 3, C], wpo_d[:], dt=bf16)

            attnT_s = wp.tile([C, B, C], bf16)      # block-diag attn^T
            K_sb = wp.tile([C, B, C], bf16)         # (attn@Wv)^T per b
            bva_s = wp.tile([C, B], f32)            # attn@bv per b
            # acc: [0:32, u, 0:32] = q-k gram; [:, u, 32] = sq-norms
            acc_s = wp.tile([2 * CO, NU, CO + 1], f32)
            nacc_s = wp.tile([2 * CO, NU, NACC], f32)
            nc.vector.memset(nacc_s[:], 0.0)
            nc.vector.memset(attnT_s[:], 0.0)
            eps_s = wp.tile([C, 1], f32)
            nc.vector.memset(eps_s[:], EPS)

            x_s = xp.tile([C, B, SLAB, W], bf16)    # x, later x_mid
            nc.sync.dma_start(x_s[:], xs.rearrange("b c h w -> c b h w"))
            yhat = yp.tile([C, B, SLAB, W], bf16)
            zt = zp.tile([C, B, SLAB, W + 2], bf16)

            cc_in = [dpp.tile([2 * CO, NS * (CO + 1)], f32, name=f"ci{b}",
                              tag=f"ci{b}") for b in range(B)]
            cc_out = [dpp.tile([2 * CO, NS * (CO + 1)], f32, name=f"co{b}",
                               tag=f"co{b}") for b in range(B)]

            # ---------------- helpers --------------------------------
            def ln_chunk(xc, outc, lp, lps, affine=False):
                """outc = (xc - mean_c(xc)) * rstd, optionally *n2w+n2b."""
                sh = [C] + list(xc.shape[1:])
                sq = lp.tile(sh, bf16, name="sq", tag="sq")
                nc.scalar.activation(sq[:], xc, AF.Square)
                s1 = lps.tile(sh, f32, tag="s1")
                nc.tensor.matmul(s1[:], ones_s[:], xc, start=True, stop=True)
                s2 = lps.tile(sh, f32, tag="s2")
                nc.tensor.matmul(s2[:], ones_s[:], sq[:],
                                 start=True, stop=True)
                mu2 = lp.tile(sh, bf16, name="mu2", tag="mu2")
                nc.scalar.activation(mu2[:], s1[:], AF.Square, scale=1.0 / C)
                var = lp.tile(sh, bf16, name="var", tag="var")
                nc.vector.scalar_tensor_tensor(
                    var[:], s2[:], 1.0 / C, mu2[:], OP.mult, OP.subtract)
                sig = lp.tile(sh, f32, name="sig", tag="sig")
                nc.scalar.activation(sig[:], var[:], AF.Sqrt,
                                     bias=eps_s[:, 0:1])
                rstd = lp.tile(sh, f32, name="rstd", tag="rstd")
                nc.vector.reciprocal_approx_fast(rstd[:], sig[:])
                dmu = lp.tile(sh, bf16, name="dmu", tag="dmu")
                nc.vector.scalar_tensor_tensor(
                    dmu[:], s1[:], -1.0 / C, xc, OP.mult, OP.add)
                nc.vector.tensor_mul(outc, dmu[:], rstd[:])
                if affine:
                    nc.scalar.activation(outc, outc, AF.Identity,
                                         scale=n2w_s[:, 0:1],
                                         bias=n2b_s[:, 0:1])

            def qk_chunk(b, si, r, ck, wqk_sl, qps, qsp, gram, nmm):
                pr, pc = nrows // r, W // r
                ppc = min(max(1, 512 // pc), pr)
                q0 = ck * ppc
                rws = min(ppc, pr - q0)
                npx = rws * pc
                u = b * NS + si
                ps = qps.tile([2 * CO, 512], f32, tag="qps")
                for ti in range(r * r):
                    dy, dx = divmod(ti, r)
                    nc.tensor.matmul(
                        ps[:, :npx],
                        wqk_sl[:, ti * 2 * CO:(ti + 1) * 2 * CO],
                        yhat[:, b, 1 + r * q0 + dy:1 + r * (q0 + rws):r,
                             dx::r],
                        start=(ti == 0), stop=(ti == r * r - 1))
                st = qsp.tile([2 * CO, 512], bf16, tag="st")
                nc.scalar.activation(st[:, :npx], ps[:, :npx], AF.Identity,
                                     bias=bqk_s[:, si:si + 1])
                nsc = qsp.tile([2 * CO, 512], bf16, tag="nsc")
                nc.scalar.activation(nsc[:, :npx], st[:, :npx], AF.Square,
                                     accum_out=nacc_s[:, u, ck:ck + 1])
                nt = npx // 128
                qkt = qsp.tile([C, 4, 2 * CO], bf16, tag="qkt")
                nc.sync.dma_start_transpose(qkt[:, 0:nt, :], st[:, :npx])
                for j in range(nt):
                    nc.tensor.matmul(
                        gram[:, si, :], qkt[:, j, CO:2 * CO],
                        qkt[:, j, 0:CO],
                        start=(nmm[si] == 0),
                        stop=(nmm[si] == NMM_U[si] - 1),
                        skip_group_check=True)
                    nmm[si] += 1

            def qk_scale_tail(b, si, qwp, qps, qsp, gram, nmm):
                r = SCALES[si]
                t0 = sum(s * s for s in SCALES[:si])
                wqk_s = qwp.tile([C, r * r * 2 * CO], bf16, name="wqk_s",
                                 tag="wqk_s")
                nc.sync.dma_start(
                    wqk_s[:],
                    bass.AP(tensor=wqk.tensor,
                            offset=wqk.offset + t0 * C * 2 * CO,
                            ap=[[2 * CO, C], [C * 2 * CO, r * r],
                                [1, 2 * CO]]))
                pr, pc = nrows // r, W // r
                ppc = min(max(1, 512 // pc), pr)
                nck = (pr + ppc - 1) // ppc
                for ck in range(nck):
                    qk_chunk(b, si, r, ck, wqk_s[:], qps, qsp, gram, nmm)

            def gram_done(b, gram):
                nc.scalar.activation(acc_s[0:CO, b * NS:(b + 1) * NS, 0:CO],
                                     gram[:], AF.Identity, bias=0.0)
                nc.vector.reduce_sum(
                    acc_s[:, b * NS:(b + 1) * NS, CO:CO + 1],
                    nacc_s[:, b * NS:(b + 1) * NS, :], axis=AX.X)

            def allreduce(b):
                sl = acc_s[:, b * NS:(b + 1) * NS, :]
                nc.sync.dma_start(cc_in[b][:],
                                  sl.rearrange("p a b -> p (a b)"))
                nc.gpsimd.collective_compute(
                    "AllReduce", OP.add,
                    replica_groups=[list(range(n_cores))],
                    ins=[cc_in[b].opt()], outs=[cc_out[b].opt()])
                nc.sync.dma_start(sl.rearrange("p a b -> p (a b)"),
                                  cc_out[b][:])

            def softmax(b, sp, sps):
                u0 = b * NS
                nrm2 = sp.tile([CO, 2, NS], f32, tag="nrm2")
                nc.scalar.activation(nrm2[:, 0, :],
                                     acc_s[0:CO, u0:u0 + NS, CO], AF.Sqrt)
                qn = sp.tile([CO, NS], f32, tag="qn")
                nc.sync.dma_start(qn[:], acc_s[CO:2 * CO, u0:u0 + NS, CO])
                nc.scalar.activation(nrm2[:, 1, :], qn[:], AF.Sqrt)
                nc.vector.tensor_scalar(nrm2[:], nrm2[:], 1e-12, None,
                                        OP.max)
                rn2 = sp.tile([CO, 2, NS], f32, tag="rn2")
                nc.vector.reciprocal(rn2[:], nrm2[:])
                nc.vector.tensor_mul(rn2[:, 1, :], rn2[:, 1, :],
                                     t_s[:, u0:u0 + NS])
                dq = sp.tile([CO, NS, CO], f32, tag="dq")
                dk = sp.tile([CO, NS, CO], f32, tag="dk")
                for si in range(NS):
                    nc.vector.tensor_scalar(dq[:, si, :], eye_s[:],
                                            rn2[:, 1, si:si + 1], None,
                                            OP.mult)
                for si in range(NS):
                    nc.vector.tensor_scalar(dk[:, si, :], eye_s[:],
                                            rn2[:, 0, si:si + 1], None,
                                            OP.mult)
                o1 = sp.tile([CO, NS, CO], f32, tag="o1")
                for si in range(NS):
                    m1 = sps.tile([CO, CO], f32, tag="m1")
                    nc.tensor.matmul(m1[:], acc_s[0:CO, u0 + si, 0:CO],
                                     dq[:, si, :], start=True, stop=True)
                    nc.scalar.activation(o1[:, si, :], m1[:], AF.Identity,
                                         bias=0.0)
                A_st = sp.tile([CO, NS, CO], f32, tag="A_st")
                for si in range(NS):
                    m2 = sps.tile([CO, CO], f32, tag="m2")
                    nc.tensor.matmul(m2[:], o1[:, si, :], dk[:, si, :],
                                     start=True, stop=True)
                    nc.scalar.activation(A_st[:, si, :], m2[:], AF.Identity,
                                         bias=0.0)
                negmax = sp.tile([CO, NS], f32, tag="negmax")
                nc.vector.reduce_max(negmax[:], A_st[:], axis=AX.X,
                                     negate=True)
                E_st = sp.tile([CO, NS, CO], f32, tag="E_st")
                for si in range(NS):
                    nc.scalar.activation(E_st[:, si, :], A_st[:, si, :],
                                         AF.Exp, bias=negmax[:, si:si + 1])
                ssum = sp.tile([CO, NS], f32, tag="ssum")
                nc.vector.reduce_sum(ssum[:], E_st[:], axis=AX.X)
                rs = sp.tile([CO, NS], f32, tag="rs")
                nc.vector.reciprocal(rs[:], ssum[:])
                at = sp.tile([CO, NS, CO], f32, tag="at")
                att = sp.tile([CO, NS, CO], f32, tag="att")
                attb = sp.tile([CO, NS, CO], bf16, tag="attb")
                for si in range(NS):
                    nc.vector.tensor_scalar(at[:, si, :], E_st[:, si, :],
                                            rs[:, si:si + 1], None, OP.mult)
                for si in range(NS):
                    nc.vector.transpose(att[:, si, :], at[:, si, :])
                for si in range(NS):
                    nc.scalar.activation(attb[:, si, :], att[:, si, :],
                                         AF.Identity, bias=0.0)
                for si in range(NS):
                    nc.sync.dma_start(
                        attnT_s[CO * si:CO * (si + 1), b,
                                CO * si:CO * (si + 1)], attb[:, si, :])

            def build_K(b, kps):
                kp = kps.tile([C, C], f32, tag="kp")
                nc.tensor.matmul(kp[:], wvn_s[:], attnT_s[:, b, :],
                                 start=True, stop=True)
                nc.scalar.activation(K_sb[:, b, :], kp[:], AF.Identity,
                                     bias=0.0)
                bp = kps.tile([C, 1], f32, tag="bp")
                nc.tensor.matmul(bp[:], attnT_s[:, b, :], bv_s[:],
                                 start=True, stop=True)
                nc.scalar.activation(bva_s[:, b:b + 1], bp[:], AF.Identity,
                                     bias=0.0)

            def chunk_2a(b, k, ap_, aps):
                yc = yhat[:, b, 2 * k:2 * k + 2, :]
                xc = x_s[:, b, 2 * k:2 * k + 2, :]
                avps = aps.tile([C, 2, W], f32, tag="a2")
                nc.tensor.matmul(avps[:], K_sb[:, b, :], yc,
                                 start=True, stop=True)
                av = ap_.tile([C, 2, W], bf16, tag="av")
                nc.scalar.activation(av[:], avps[:], AF.Identity,
                                     bias=bva_s[:, b:b + 1])
                pjps = aps.tile([C, 2, W], f32, tag="a2")
                nc.tensor.matmul(pjps[:], wpj_s[:], av[:],
                                 start=True, stop=True)
                tA = ap_.tile([C, 2, W], f32, tag="tA")
                nc.vector.scalar_tensor_tensor(
                    tA[:], pjps[:], n1b_s[:, 0:1], xc, OP.add, OP.add)
                nc.vector.scalar_tensor_tensor(
                    xc, yc, n1w_s[:, 0:1], tA[:], OP.mult, OP.add)

            def chunk_ln2(b, k, lp2, lps2):
                ln_chunk(x_s[:, b, 2 * k:2 * k + 2, :],
                         zt[:, b, 2 * k:2 * k + 2, 1:W + 1],
                         lp2, lps2, affine=True)
                if k == 0:
                    nc.vector.tensor_scalar(
                        zt[:, b, 0, 1:W + 1], zt[:, b, 0, 1:W + 1],
                        hm_s[:, 0:1], None, OP.mult)
                if k == NCH - 1:
                    nc.vector.tensor_scalar(
                        zt[:, b, SLAB - 1, 1:W + 1],
                        zt[:, b, SLAB - 1, 1:W + 1],
                        hm_s[:, 1:2], None, OP.mult)

            def chunk_ffn(b, j, fps, ops_, gp, outp):
                ops = ops_.tile([C, 2, W], f32, tag="a2")
                for p in range(3):
                    mh = MH[p]
                    f1 = fps.tile([C, 2, W], f32, tag="f1", bufs=2)
                    f2 = fps.tile([C, 2, W], f32, tag="f2")
                    for ti in range(9):
                        dy, dx = divmod(ti, 3)
                        rhs = zt[:, b, 2 * j + dy:2 * j + dy + 2,
                                 dx:W + dx]
                        c1 = 128 * p
                        nc.tensor.matmul(
                            f1[:mh], wf_s[:, 2 * HID * ti + c1:
                                          2 * HID * ti + c1 + mh],
                            rhs, start=(ti == 0), stop=(ti == 8))
                        c2 = HID + 128 * p
                        nc.tensor.matmul(
                            f2[:mh], wf_s[:, 2 * HID * ti + c2:
                                          2 * HID * ti + c2 + mh],
                            rhs, start=(ti == 0), stop=(ti == 8))
                    g1 = gp.tile([C, 2, W], bf16, tag="g1")
                    nc.scalar.activation(g1[:mh], f1[:mh], AF.Gelu)
                    g = gp.tile([C, 2, W], bf16, tag="g")
                    nc.vector.tensor_mul(g[:mh], g1[:mh], f2[:mh])
                    nc.tensor.matmul(ops[:], wpo_s[:mh, p, :], g[:mh],
                                     start=(p == 0), stop=(p == 2))
                o_sb = outp.tile([C, 2, W], f32, tag="o_sb")
                nc.vector.tensor_add(
                    o_sb[:], x_s[:, b, 2 * j + 1:2 * j + 3, :], ops[:])
                nc.sync.dma_start(out_d[b, :, 2 * j:2 * j + 2, :], o_sb[:])

            # ==========================================================
            # Phase A: b0 LN1 + qk, AR#0, b1 LN1 prelude
            # ==========================================================
            with tc.tile_pool(name="lnA", bufs=4) as lpA, \
                 tc.tile_pool(name="lnAps", bufs=2, space="PSUM") as lpsA, \
                 tc.tile_pool(name="qkwA", bufs=1) as qwpA, \
                 tc.tile_pool(name="qkstA", bufs=4) as qspA, \
                 tc.tile_pool(name="qkpsA", bufs=2, space="PSUM") as qpsA, \
                 tc.tile_pool(name="grpsA", bufs=1, space="PSUM") as gpsA:
                wqk0_s = qwpA.tile([C, 2 * CO], bf16, name="wqk0_s",
                                   tag="wqk0")
                nc.sync.dma_start(wqk0_s[:], wqk[0])
                gramA = gpsA.tile([CO, NS, CO], f32)
                nmmA = {si: 0 for si in range(NS)}
                for m in range(NCH + 1):
                    if m < NCH:
                        ln_chunk(x_s[:, 0, 2 * m:2 * m + 2, :],
                                 yhat[:, 0, 2 * m:2 * m + 2, :], lpA, lpsA)
                    if m >= 2:
                        qk_chunk(0, 0, 1, m - 2, wqk0_s[:], qpsA, qspA,
                                 gramA[:], nmmA)
                for si in range(1, NS):
                    qk_scale_tail(0, si, qwpA, qpsA, qspA, gramA[:], nmmA)
                gram_done(0, gramA[:])
                allreduce(0)
                # prelude: b1 LN1 chunks hide the collective latency
                for m in range(PRE):
                    ln_chunk(x_s[:, 1, 2 * m:2 * m + 2, :],
                             yhat[:, 1, 2 * m:2 * m + 2, :], lpA, lpsA)

            with tc.tile_pool(name="smx0", bufs=1) as sp0, \
                 tc.tile_pool(name="smxps0", bufs=2, space="PSUM") as sps0:
                softmax(0, sp0, sps0)
                build_K(0, sps0)

            # ==========================================================
            # MEGA phase: b0 main loop with b1 prep in the slack
            # PSUM: s1+s2(2) qps(1) gram(1) a2(1) f1x2(2) f2(1) = 8
            # ==========================================================
            with tc.tile_pool(name="lnM", bufs=2) as lpM, \
                 tc.tile_pool(name="lnMps", bufs=1, space="PSUM") as lpsM, \
                 tc.tile_pool(name="qkwM", bufs=1) as qwpM, \
                 tc.tile_pool(name="qkstM", bufs=4) as qspM, \
                 tc.tile_pool(name="qkpsM", bufs=1, space="PSUM") as qpsM, \
                 tc.tile_pool(name="grpsM", bufs=1, space="PSUM") as gpsM, \
                 tc.tile_pool(name="p2aM", bufs=2) as apM, \
                 tc.tile_pool(name="p2aMps", bufs=1, space="PSUM") as apsM, \
                 tc.tile_pool(name="fpsM", bufs=1, space="PSUM") as fpsM, \
                 tc.tile_pool(name="gpM", bufs=2) as gpM, \
                 tc.tile_pool(name="outM", bufs=3) as outM:
                nc.vector.memset(zt[:, :, :, 0:1], 0.0)
                nc.vector.memset(zt[:, :, :, W + 1:W + 2], 0.0)
                gramM = gpsM.tile([CO, NS, CO], f32)
                nmmM = {si: 0 for si in range(NS)}

                # b1 side-work schedule: iteration -> list of thunks
                side = {k: [] for k in range(NCH + 1)}
                for m in range(PRE, NCH):          # LN1 tail: 5 chunks
                    side[m - PRE].append(
                        ("ln", m))
                qk_iter = []
                for ck in range(16):
                    qk_iter.append(("qk0", ck))
                for si in range(1, NS):
                    qk_iter.append(("qks", si))
                # 2 qk items per iteration starting at k=3
                for i, item in enumerate(qk_iter):
                    side[3 + i // 2].append(item)
                last_qk = 3 + (len(qk_iter) - 1) // 2
                side[last_qk + 1].append(("ar", None))

                wqk0M = qwpM.tile([C, 2 * CO], bf16, name="wqk0M",
                                  tag="wqk0")
                nc.sync.dma_start(wqk0M[:], wqk[0])

                for k in range(NCH + 1):
                    if k < NCH:
                        chunk_2a(0, k, apM, apsM)
                        chunk_ln2(0, k, lpM, lpsM)
                    for kind, arg in side[k]:
                        if kind == "ln":
                            ln_chunk(x_s[:, 1, 2 * arg:2 * arg + 2, :],
                                     yhat[:, 1, 2 * arg:2 * arg + 2, :],
                                     lpM, lpsM)
                        elif kind == "qk0":
                            qk_chunk(1, 0, 1, arg, wqk0M[:], qpsM, qspM,
                                     gramM[:], nmmM)
                        elif kind == "qks":
                            qk_scale_tail(1, arg, qwpM, qpsM, qspM,
                                          gramM[:], nmmM)
                        elif kind == "ar":
                            gram_done(1, gramM[:])
                            allreduce(1)
                    if k >= 2 and k - 2 < NFF:
                        chunk_ffn(0, k - 2, fpsM, apsM, gpM, outM)

            with tc.tile_pool(name="smx1", bufs=1) as sp1, \
                 tc.tile_pool(name="smxps1", bufs=2, space="PSUM") as sps1:
                softmax(1, sp1, sps1)
                build_K(1, sps1)

            # ==========================================================
            # b1 main loop
            # ==========================================================
            with tc.tile_pool(name="ln2", bufs=2) as lp2, \
                 tc.tile_pool(name="ln2ps", bufs=1, space="PSUM") as lps2, \
                 tc.tile_pool(name="p2a", bufs=2) as ap_, \
                 tc.tile_pool(name="p2aps", bufs=2, space="PSUM") as aps, \
                 tc.tile_pool(name="fps", bufs=1, space="PSUM") as fps, \
                 tc.tile_pool(name="gp", bufs=2) as gp, \
                 tc.tile_pool(name="outp", bufs=3) as outp:
                for k in range(NCH + 1):
                    if k < NCH:
                        chunk_2a(1, k, ap_, aps)
                        chunk_ln2(1, k, lp2, lps2)
                    if k >= 2 and k - 2 < NFF:
                        chunk_ffn(1, k - 2, fps, aps, gp, outp)

    nc.compile()
    return nc


# ---------------------------------------------------------------------------
# host side
# ---------------------------------------------------------------------------

def _prep_inputs(inputs, nrows, n_cores):
    import ml_dtypes
    bf = ml_dtypes.bfloat16
    H = nrows * n_cores
    x = np.asarray(inputs["x"], np.float32)
    n1w = np.asarray(inputs["n1w"], np.float32)
    n1b = np.asarray(inputs["n1b"], np.float32)
    n2w = np.asarray(inputs["n2w"], np.float32)
    n2b = np.asarray(inputs["n2b"], np.float32)

    wqk_taps = np.zeros((NTAPS, C, 2 * CO), np.float32)
    bqk = np.zeros((2 * CO, NS), np.float32)
    ti = 0
    for si, r in enumerate(SCALES):
        wqkw = np.asarray(inputs[f"wqk{si}"], np.float32)  # [64,128,r,r]
        wqkf = wqkw * n1w[None, :, None, None]
        bqk[:, si] = np.einsum("ocyx,c->o", wqkw, n1b)
        for dy in range(r):
            for dx in range(r):
                wqk_taps[ti] = wqkf[:, :, dy, dx].T
                ti += 1

    wv_cat = np.concatenate([np.asarray(inputs[f"wv{i}"],
                                        np.float32)[:, :, 0, 0]
                             for i in range(NS)], axis=0)      # [vch, in]
    bv_cat = np.concatenate([np.asarray(inputs[f"bv{i}"], np.float32)
                             for i in range(NS)])
    bv_all = (wv_cat @ n1b + bv_cat).astype(np.float32)
    wvn = (wv_cat * n1w[None, :])                              # [vch, in]

    wpj_t = np.asarray(inputs["wproj"], np.float32)[:, :, 0, 0].T.copy()

    wpi = np.asarray(inputs["wpi"], np.float32)[:, :, 0, 0]    # [680,128]
    wdw = np.asarray(inputs["wdw"], np.float32)[:, 0]          # [680,3,3]
    wf = np.zeros((9, C, 2 * HID), np.float32)
    for ti in range(9):
        dy, dx = divmod(ti, 3)
        wf[ti] = (wpi * wdw[:, dy, dx][:, None]).T             # [128,680]

    wpo = np.asarray(inputs["wpo"], np.float32)[:, :, 0, 0]    # [128,340]
    wpo_p = np.zeros((C, 3, C), np.float32)
    for p in range(3):
        mh = min(C, HID - C * p)
        wpo_p[:mh, p, :] = wpo[:, C * p:C * p + mh].T

    tv = np.zeros((1, NU), np.float32)
    for si in range(NS):
        for b in range(B):
            tv[0, b * NS + si] = float(
                np.asarray(inputs[f"t{si}"]).reshape(-1)[0])
    eye = np.eye(CO, dtype=np.float32)

    shared = {
        "wqk": wqk_taps.astype(bf), "bqk": bqk,
        "wvn": wvn.astype(bf), "bv": bv_all.reshape(C, 1).astype(bf),
        "wpj": wpj_t.astype(bf),
        "n1w": n1w.reshape(C, 1), "n1b": n1b.reshape(C, 1),
        "n2w": n2w.reshape(C, 1), "n2b": n2b.reshape(C, 1),
        "wf": wf.astype(bf), "wpo": wpo_p.astype(bf),
        "tvec": tv, "eye": eye,
        "ones": np.ones((C, C), np.float32).astype(bf),
    }

    in_maps = []
    for i in range(n_cores):
        r0 = nrows * i
        slab = np.zeros((B, C, nrows + 2, W), np.float32)
        lo, hi = r0 - 1, r0 + nrows + 1
        slo, shi = max(lo, 0), min(hi, H)
        slab[:, :, slo - lo:shi - lo, :] = x[:, :, slo:shi, :]
        m = {"xs": slab.astype(bf),
             "hmask": np.array([[1.0 if i > 0 else 0.0,
                                 1.0 if i < n_cores - 1 else 0.0]],
                               np.float32)}
        m.update(shared)
        in_maps.append(m)
    return in_maps


def _run(nrows, n_cores, in_maps, trace=False):
    from concourse.bass_utils import run_bass_kernel_spmd
    key = (nrows, n_cores)
    if key not in _CACHE:
        _CACHE[key] = _build(nrows, n_cores)
    nc = _CACHE[key]
    return run_bass_kernel_spmd(nc, in_maps, core_ids=list(range(n_cores)),
                                trace=trace)


def run_sharded(inputs, nrows=32, n_cores=8, trace=False):
    in_maps = _prep_inputs(inputs, nrows, n_cores)
    res = _run(nrows, n_cores, in_maps, trace=trace)
    H = nrows * n_cores
    out = np.zeros((B, C, H, W), np.float32)
    for i in range(n_cores):
        out[:, :, nrows * i:nrows * (i + 1), :] = res.results[i]["out"]
    return out, res


def kernel(**inputs):
    out, _ = run_sharded(inputs, nrows=32, n_cores=8)
    return out


# revision 27
# speedup vs baseline: 1.4309x; 1.0066x over previous
"""Trainium2 Bass kernel for MultiScaleChannelTransformerBlock.

kernel(**inputs) takes the FULL inputs (as produced by setup_inputs())
and returns the FULL output [2, 128, 256, 256] float32.

Sharding: spatial over H across 8 NeuronCores (32 rows each, plus a
1-row halo on each side, host-padded).  Cross-core communication is
one small AllReduce per batch image (attention q-k Gram block + q/k
squared norms).

Pipeline (per core), designed so the PE never idles after softmax:
  A:  LN1(b0) with the scale-1 qk conv interleaved, then scales 2/4/8
      -> gram(b0)+norms(b0); AllReduce#0 issued, hidden under a
      prelude of LN1(b1) chunks.
  sm0/K0: batched softmax for b0; wv and bv are folded through the
      attention on-device: K = (attn@Wv)^T, bv_att = attn@bv, so the
      v conv disappears entirely.
  MEGA: the b0 main loop (attn-apply + wproj + residuals, LN2 into a
      persistent padded z slab, folded-FFN trailing 2 chunks behind)
      with the REMAINING b1 prep work (LN1 tail, all qk convs, gram,
      AllReduce#1) slotted into its DVE/ACT slack.  The PE is dense
      on FFN matmuls throughout.
  sm1/K1, then the b1 main loop.

All matmul operands are bf16 (fp32 moving operands stream at half
rate on the PE); accumulation is fp32 in PSUM.
"""

import numpy as np

B = 2
C = 128
CO = 32
HID = 340
W = 256
SCALES = [1, 2, 4, 8]
NS = len(SCALES)
NU = NS * B
NTAPS = sum(r * r for r in SCALES)  # 85
EPS = 1e-5

_CACHE = {}


def _build(nrows, n_cores):
    import concourse.bass as bass
    import concourse.tile as tile
    from concourse import bacc, mybir

    f32 = mybir.dt.float32
    bf16 = mybir.dt.bfloat16
    AF = mybir.ActivationFunctionType
    OP = mybir.AluOpType
    AX = mybir.AxisListType

    assert nrows % 2 == 0
    SLAB = nrows + 2
    NCH = SLAB // 2                   # ln/2a chunks per batch (17)
    NFF = nrows // 2                  # ffn chunks per batch (16)
    seg = {r: (nrows // r) * (W // r) for r in SCALES}
    MH = [C, C, HID - 2 * C]          # FFN hidden blocks: 128,128,84
    NACC = 16                         # norm accum slots per unit
    NMM_U = {0: 64, 1: 16, 2: 4, 3: 1}  # gram matmuls per (scale)
    PRE = 12                          # b1 LN1 chunks issued under AR#0

    nc = bacc.Bacc("TRN2", target_bir_lowering=False, debug=False,
                   num_devices=n_cores)

    def din(name, shape, dt=f32):
        return nc.dram_tensor(name, shape, dt, kind="ExternalInput").ap()

    xs = din("xs", [B, C, SLAB, W], bf16)
    wqk = din("wqk", [NTAPS, C, 2 * CO], bf16)
    bqk_d = din("bqk", [2 * CO, NS])
    wvn_d = din("wvn", [C, C], bf16)     # Wv*n1w, [v_ch, in]
    bv_d = din("bv", [C, 1], bf16)       # Wv@n1b + bv, [v_ch, 1]
    wpj_d = din("wpj", [C, C], bf16)
    n1w_d = din("n1w", [C, 1])
    n1b_d = din("n1b", [C, 1])
    n2w_d = din("n2w", [C, 1])
    n2b_d = din("n2b", [C, 1])
    wf_d = din("wf", [9, C, 2 * HID], bf16)
    wpo_d = din("wpo", [C, 3, C], bf16)
    tvec = din("tvec", [1, NU])          # u = b*NS + si
    eye_d = din("eye", [CO, CO])
    ones_d = din("ones", [C, C], bf16)
    hmask = din("hmask", [1, 2])
    out_d = nc.dram_tensor("out", [B, C, nrows, W], f32,
                           kind="ExternalOutput").ap()

    with tile.TileContext(nc) as tc:
        with tc.tile_pool(name="wpers", bufs=1) as wp, \
             tc.tile_pool(name="xbig", bufs=1) as xp, \
             tc.tile_pool(name="ybig", bufs=1) as yp, \
             tc.tile_pool(name="zbig", bufs=1) as zp, \
             tc.tile_pool(name="ccd", bufs=1, space="DRAM") as dpp:

            def load(nm, shape, src, dt=f32):
                t = wp.tile(shape, dt, name=nm, tag=nm)
                nc.sync.dma_start(t[:], src)
                return t

            wvn_s = load("wvn_s", [C, C], wvn_d[:], dt=bf16)
            wpj_s = load("wpj_s", [C, C], wpj_d[:], dt=bf16)
            n1w_s = load("n1w_s", [C, 1], n1w_d[:])
            n1b_s = load("n1b_s", [C, 1], n1b_d[:])
            n2w_s = load("n2w_s", [C, 1], n2w_d[:])
            n2b_s = load("n2b_s", [C, 1], n2b_d[:])
            bv_s = load("bv_s", [C, 1], bv_d[:], dt=bf16)
            bqk_s = load("bqk_s", [2 * CO, NS], bqk_d[:])
            ones_s = load("ones_s", [C, C], ones_d[:], dt=bf16)
            eye_s = load("eye_s", [CO, CO], eye_d[:])
            t_s = load("t_s", [CO, NU],
                       bass.AP(tensor=tvec.tensor, offset=tvec.offset,
                               ap=[[0, CO], [1, NU]]))
            hm_s = load("hm_s", [C, 2],
                        bass.AP(tensor=hmask.tensor, offset=hmask.offset,
                                ap=[[0, C], [1, 2]]))
            wf_s = wp.tile([C, 9 * 2 * HID], bf16, name="wf_s", tag="wf_s")
            nc.sync.dma_start(
                wf_s[:],
                bass.AP(tensor=wf_d.tensor, offset=wf_d.offset,
                        ap=[[2 * HID, C], [C * 2 * HID, 9], [1, 2 * HID]]))
            wpo_s = load("wpo_s", [C, 3, C], wpo_d[:], dt=bf16)

            attnT_s = wp.tile([C, B, C], bf16)      # block-diag attn^T
            K_sb = wp.tile([C, B, C], bf16)         # (attn@Wv)^T per b
            bva_s = wp.tile([C, B], f32)            # attn@bv per b
            # acc: [0:32, u, 0:32] = q-k gram; [:, u, 32] = sq-norms
            acc_s = wp.tile([2 * CO, NU, CO + 1], f32)
            nacc_s = wp.tile([2 * CO, NU, NACC], f32)
            nc.vector.memset(nacc_s[:], 0.0)
            nc.vector.memset(attnT_s[:], 0.0)
            eps_s = wp.tile([C, 1], f32)
            nc.vector.memset(eps_s[:], EPS)

            x_s = xp.tile([C, B, SLAB, W], bf16)    # x, later x_mid
            # split the input DMA so LN1 can start before the whole
            # slab has landed
            for (bb, r0, r1) in [(0, 0, 12), (0, 12, 24), (0, 24, SLAB),
                                 (1, 0, 17), (1, 17, SLAB)]:
                nc.sync.dma_start(x_s[:, bb, r0:r1, :],
                                  xs[bb, :, r0:r1, :])
            yhat = yp.tile([C, B, SLAB, W], bf16)
            zt = zp.tile([C, B, SLAB, W + 2], bf16)

            cc_in = [dpp.tile([2 * CO, NS * (CO + 1)], f32, name=f"ci{b}",
                              tag=f"ci{b}") for b in range(B)]
            cc_out = [dpp.tile([2 * CO, NS * (CO + 1)], f32, name=f"co{b}",
                               tag=f"co{b}") for b in range(B)]

            # ---------------- helpers --------------------------------
            def ln_chunk(xc, outc, lp, lps, affine=False):
                """outc = (xc - mean_c(xc)) * rstd, optionally *n2w+n2b."""
                sh = [C] + list(xc.shape[1:])
                sq = lp.tile(sh, bf16, name="sq", tag="sq")
                nc.scalar.activation(sq[:], xc, AF.Square)
                s1 = lps.tile(sh, f32, tag="s1")
                nc.tensor.matmul(s1[:], ones_s[:], xc, start=True, stop=True)
                s2 = lps.tile(sh, f32, tag="s2")
                nc.tensor.matmul(s2[:], ones_s[:], sq[:],
                                 start=True, stop=True)
                mu2 = lp.tile(sh, f32, name="mu2", tag="mu2")
                nc.scalar.activation(mu2[:], s1[:], AF.Square, scale=1.0 / C)
                var = lp.tile(sh, f32, name="var", tag="var")
                nc.vector.scalar_tensor_tensor(
                    var[:], s2[:], 1.0 / C, mu2[:], OP.mult, OP.subtract)
                sig = lp.tile(sh, f32, name="sig", tag="sig")
                nc.scalar.activation(sig[:], var[:], AF.Sqrt,
                                     bias=eps_s[:, 0:1])
                rstd = lp.tile(sh, f32, name="rstd", tag="rstd")
                nc.vector.reciprocal_approx_fast(rstd[:], sig[:])
                dmu = lp.tile(sh, bf16, name="dmu", tag="dmu")
                nc.vector.scalar_tensor_tensor(
                    dmu[:], s1[:], -1.0 / C, xc, OP.mult, OP.add)
                nc.vector.tensor_mul(outc, dmu[:], rstd[:])
                if affine:
                    nc.scalar.activation(outc, outc, AF.Identity,
                                         scale=n2w_s[:, 0:1],
                                         bias=n2b_s[:, 0:1])

            def qk_chunk(b, si, r, ck, wqk_sl, qps, qsp, gram, nmm):
                pr, pc = nrows // r, W // r
                ppc = min(max(1, 512 // pc), pr)
                q0 = ck * ppc
                rws = min(ppc, pr - q0)
                npx = rws * pc
                u = b * NS + si
                ps = qps.tile([2 * CO, 512], f32, tag="qps")
                for ti in range(r * r):
                    dy, dx = divmod(ti, r)
                    nc.tensor.matmul(
                        ps[:, :npx],
                        wqk_sl[:, ti * 2 * CO:(ti + 1) * 2 * CO],
                        yhat[:, b, 1 + r * q0 + dy:1 + r * (q0 + rws):r,
                             dx::r],
                        start=(ti == 0), stop=(ti == r * r - 1))
                st = qsp.tile([2 * CO, 512], bf16, tag="st")
                nc.scalar.activation(st[:, :npx], ps[:, :npx], AF.Identity,
                                     bias=bqk_s[:, si:si + 1])
                nsc = qsp.tile([2 * CO, 512], bf16, tag="nsc")
                nc.scalar.activation(nsc[:, :npx], st[:, :npx], AF.Square,
                                     accum_out=nacc_s[:, u, ck:ck + 1])
                nt = npx // 128
                qkt = qsp.tile([C, 4, 2 * CO], bf16, tag="qkt")
                nc.sync.dma_start_transpose(qkt[:, 0:nt, :], st[:, :npx])
                for j in range(nt):
                    nc.tensor.matmul(
                        gram[:, si, :], qkt[:, j, CO:2 * CO],
                        qkt[:, j, 0:CO],
                        start=(nmm[si] == 0),
                        stop=(nmm[si] == NMM_U[si] - 1),
                        skip_group_check=True)
                    nmm[si] += 1

            def qk_scale_tail(b, si, qwp, qps, qsp, gram, nmm):
                r = SCALES[si]
                t0 = sum(s * s for s in SCALES[:si])
                wqk_s = qwp.tile([C, r * r * 2 * CO], bf16, name="wqk_s",
                                 tag="wqk_s")
                nc.sync.dma_start(
                    wqk_s[:],
                    bass.AP(tensor=wqk.tensor,
                            offset=wqk.offset + t0 * C * 2 * CO,
                            ap=[[2 * CO, C], [C * 2 * CO, r * r],
                                [1, 2 * CO]]))
                pr, pc = nrows // r, W // r
                ppc = min(max(1, 512 // pc), pr)
                nck = (pr + ppc - 1) // ppc
                for ck in range(nck):
                    qk_chunk(b, si, r, ck, wqk_s[:], qps, qsp, gram, nmm)

            def gram_done(b, gram):
                nc.scalar.activation(acc_s[0:CO, b * NS:(b + 1) * NS, 0:CO],
                                     gram[:], AF.Identity, bias=0.0)
                nc.vector.reduce_sum(
                    acc_s[:, b * NS:(b + 1) * NS, CO:CO + 1],
                    nacc_s[:, b * NS:(b + 1) * NS, :], axis=AX.X)

            def allreduce(b):
                sl = acc_s[:, b * NS:(b + 1) * NS, :]
                nc.sync.dma_start(cc_in[b][:],
                                  sl.rearrange("p a b -> p (a b)"))
                nc.gpsimd.collective_compute(
                    "AllReduce", OP.add,
                    replica_groups=[list(range(n_cores))],
                    ins=[cc_in[b].opt()], outs=[cc_out[b].opt()])
                nc.sync.dma_start(sl.rearrange("p a b -> p (a b)"),
                                  cc_out[b][:])

            def softmax(b, sp, mk_m):
                u0 = b * NS
                nrm2 = sp.tile([CO, 2, NS], f32, tag="nrm2")
                nc.scalar.activation(nrm2[:, 0, :],
                                     acc_s[0:CO, u0:u0 + NS, CO], AF.Sqrt)
                qn = sp.tile([CO, NS], f32, tag="qn")
                nc.sync.dma_start(qn[:], acc_s[CO:2 * CO, u0:u0 + NS, CO])
                nc.scalar.activation(nrm2[:, 1, :], qn[:], AF.Sqrt)
                nc.vector.tensor_scalar(nrm2[:], nrm2[:], 1e-12, None,
                                        OP.max)
                rn2 = sp.tile([CO, 2, NS], f32, tag="rn2")
                nc.vector.reciprocal(rn2[:], nrm2[:])
                nc.vector.tensor_mul(rn2[:, 1, :], rn2[:, 1, :],
                                     t_s[:, u0:u0 + NS])
                dq = sp.tile([CO, NS, CO], f32, tag="dq")
                dk = sp.tile([CO, NS, CO], f32, tag="dk")
                for si in range(NS):
                    nc.vector.tensor_scalar(dq[:, si, :], eye_s[:],
                                            rn2[:, 1, si:si + 1], None,
                                            OP.mult)
                for si in range(NS):
                    nc.vector.tensor_scalar(dk[:, si, :], eye_s[:],
                                            rn2[:, 0, si:si + 1], None,
                                            OP.mult)
                o1 = sp.tile([CO, NS, CO], f32, tag="o1")
                for si in range(NS):
                    m1 = mk_m()
                    nc.tensor.matmul(m1, acc_s[0:CO, u0 + si, 0:CO],
                                     dq[:, si, :], start=True, stop=True)
                    nc.scalar.activation(o1[:, si, :], m1, AF.Identity,
                                         bias=0.0)
                A_st = sp.tile([CO, NS, CO], f32, tag="A_st")
                for si in range(NS):
                    m2 = mk_m()
                    nc.tensor.matmul(m2, o1[:, si, :], dk[:, si, :],
                                     start=True, stop=True)
                    nc.scalar.activation(A_st[:, si, :], m2, AF.Identity,
                                         bias=0.0)
                negmax = sp.tile([CO, NS], f32, tag="negmax")
                nc.vector.reduce_max(negmax[:], A_st[:], axis=AX.X,
                                     negate=True)
                E_st = sp.tile([CO, NS, CO], f32, tag="E_st")
                for si in range(NS):
                    nc.scalar.activation(E_st[:, si, :], A_st[:, si, :],
                                         AF.Exp, bias=negmax[:, si:si + 1])
                ssum = sp.tile([CO, NS], f32, tag="ssum")
                nc.vector.reduce_sum(ssum[:], E_st[:], axis=AX.X)
                rs = sp.tile([CO, NS], f32, tag="rs")
                nc.vector.reciprocal(rs[:], ssum[:])
                at = sp.tile([CO, NS, CO], f32, tag="at")
                att = sp.tile([CO, NS, CO], f32, tag="att")
                attb = sp.tile([CO, NS, CO], bf16, tag="attb")
                for si in range(NS):
                    nc.vector.tensor_scalar(at[:, si, :], E_st[:, si, :],
                                            rs[:, si:si + 1], None, OP.mult)
                for si in range(NS):
                    nc.vector.transpose(att[:, si, :], at[:, si, :])
                for si in range(NS):
                    nc.scalar.activation(attb[:, si, :], att[:, si, :],
                                         AF.Identity, bias=0.0)
                for si in range(NS):
                    nc.sync.dma_start(
                        attnT_s[CO * si:CO * (si + 1), b,
                                CO * si:CO * (si + 1)], attb[:, si, :])

            def build_K(b, mk_kp, mk_bp):
                kp = mk_kp()
                nc.tensor.matmul(kp, wvn_s[:], attnT_s[:, b, :],
                                 start=True, stop=True)
                nc.scalar.activation(K_sb[:, b, :], kp, AF.Identity,
                                     bias=0.0)
                bp = mk_bp()
                nc.tensor.matmul(bp, attnT_s[:, b, :], bv_s[:],
                                 start=True, stop=True)
                nc.scalar.activation(bva_s[:, b:b + 1], bp, AF.Identity,
                                     bias=0.0)

            def chunk_2a(b, k, ap_, aps):
                yc = yhat[:, b, 2 * k:2 * k + 2, :]
                xc = x_s[:, b, 2 * k:2 * k + 2, :]
                avps = aps.tile([C, 2, W], f32, tag="a2")
                nc.tensor.matmul(avps[:], K_sb[:, b, :], yc,
                                 start=True, stop=True)
                av = ap_.tile([C, 2, W], bf16, tag="av")
                nc.scalar.activation(av[:], avps[:], AF.Identity,
                                     bias=bva_s[:, b:b + 1])
                pjps = aps.tile([C, 2, W], f32, tag="a2")
                nc.tensor.matmul(pjps[:], wpj_s[:], av[:],
                                 start=True, stop=True)
                tA = ap_.tile([C, 2, W], f32, tag="tA")
                nc.vector.scalar_tensor_tensor(
                    tA[:], pjps[:], n1b_s[:, 0:1], xc, OP.add, OP.add)
                nc.vector.scalar_tensor_tensor(
                    xc, yc, n1w_s[:, 0:1], tA[:], OP.mult, OP.add)

            def chunk_ln2(b, k, lp2, lps2):
                ln_chunk(x_s[:, b, 2 * k:2 * k + 2, :],
                         zt[:, b, 2 * k:2 * k + 2, 1:W + 1],
                         lp2, lps2, affine=True)
                if k == 0:
                    nc.vector.tensor_scalar(
                        zt[:, b, 0, 1:W + 1], zt[:, b, 0, 1:W + 1],
                        hm_s[:, 0:1], None, OP.mult)
                if k == NCH - 1:
                    nc.vector.tensor_scalar(
                        zt[:, b, SLAB - 1, 1:W + 1],
                        zt[:, b, SLAB - 1, 1:W + 1],
                        hm_s[:, 1:2], None, OP.mult)

            def chunk_ffn(b, j, fps, ops_, gp, outp):
                ops = ops_.tile([C, 2, W], f32, tag="a2")
                for p in range(3):
                    mh = MH[p]
                    f1 = fps.tile([C, 2, W], f32, tag="f1", bufs=2)
                    f2 = fps.tile([C, 2, W], f32, tag="f2")
                    for ti in range(9):
                        dy, dx = divmod(ti, 3)
                        rhs = zt[:, b, 2 * j + dy:2 * j + dy + 2,
                                 dx:W + dx]
                        c1 = 128 * p
                        nc.tensor.matmul(
                            f1[:mh], wf_s[:, 2 * HID * ti + c1:
                                          2 * HID * ti + c1 + mh],
                            rhs, start=(ti == 0), stop=(ti == 8))
                        c2 = HID + 128 * p
                        nc.tensor.matmul(
                            f2[:mh], wf_s[:, 2 * HID * ti + c2:
                                          2 * HID * ti + c2 + mh],
                            rhs, start=(ti == 0), stop=(ti == 8))
                    g1 = gp.tile([C, 2, W], bf16, tag="g1")
                    nc.scalar.activation(g1[:mh], f1[:mh], AF.Gelu)
                    g = gp.tile([C, 2, W], bf16, tag="g")
                    nc.vector.tensor_mul(g[:mh], g1[:mh], f2[:mh])
                    nc.tensor.matmul(ops[:], wpo_s[:mh, p, :], g[:mh],
                                     start=(p == 0), stop=(p == 2))
                o_sb = outp.tile([C, 2, W], f32, tag="o_sb")
                nc.vector.tensor_add(
                    o_sb[:], x_s[:, b, 2 * j + 1:2 * j + 3, :], ops[:])
                nc.sync.dma_start(out_d[b, :, 2 * j:2 * j + 2, :], o_sb[:])

            # ==========================================================
            # Phase A: b0 LN1 + qk, AR#0, b1 LN1+qk prelude
            # ==========================================================
            nmmA = {si: 0 for si in range(NS)}
            nmmM = {si: 0 for si in range(NS)}
            with tc.tile_pool(name="grall", bufs=1, space="PSUM") as gr_p:
                gram_all = gr_p.tile([CO, B, NS, CO], f32, name="gram_all",
                                     tag="ga")
                gramA = gram_all[:, 0]
                gramM = gram_all[:, 1]
                with tc.tile_pool(name="lnA", bufs=4) as lpA, \
                     tc.tile_pool(name="lnAps", bufs=3,
                                  space="PSUM") as lpsA, \
                     tc.tile_pool(name="qkwA", bufs=1) as qwpA, \
                     tc.tile_pool(name="qkstA", bufs=4) as qspA, \
                     tc.tile_pool(name="qkpsA", bufs=1,
                                  space="PSUM") as qpsA:
                    wqk0_s = qwpA.tile([C, 2 * CO], bf16, name="wqk0_s",
                                       tag="wqk0")
                    nc.sync.dma_start(wqk0_s[:], wqk[0])
                    for m in range(NCH + 1):
                        if m < NCH:
                            ln_chunk(x_s[:, 0, 2 * m:2 * m + 2, :],
                                     yhat[:, 0, 2 * m:2 * m + 2, :],
                                     lpA, lpsA)
                        if m >= 2:
                            qk_chunk(0, 0, 1, m - 2, wqk0_s[:], qpsA, qspA,
                                     gramA[:], nmmA)
                    for si in range(1, NS):
                        qk_scale_tail(0, si, qwpA, qpsA, qspA, gramA[:],
                                      nmmA)
                    gram_done(0, gramA[:])
                    allreduce(0)
                    # prelude hides the collective latency: all of b1 LN1
                    for m in range(NCH):
                        ln_chunk(x_s[:, 1, 2 * m:2 * m + 2, :],
                                 yhat[:, 1, 2 * m:2 * m + 2, :], lpA, lpsA)

                with tc.tile_pool(name="smx0", bufs=1) as sp0, \
                     tc.tile_pool(name="smxps0", bufs=2,
                                  space="PSUM") as sps0:
                    softmax(0, sp0,
                            lambda: sps0.tile([CO, CO], f32, name="m0", tag="m")[:])
                    build_K(0,
                            lambda: sps0.tile([C, C], f32, name="kp0", tag="kp")[:],
                            lambda: sps0.tile([C, 1], f32, name="bp0", tag="bp")[:])

                # ======================================================
                # MEGA phase: b0 main loop with b1 prep in the slack
                # PSUM: gram(1) s1+s2(2) qps(1) a2(1) f1x2(2) f2(1) = 8
                # ======================================================
                with tc.tile_pool(name="lnM", bufs=2) as lpM, \
                     tc.tile_pool(name="lnMps", bufs=1,
                                  space="PSUM") as lpsM, \
                     tc.tile_pool(name="qkwM", bufs=1) as qwpM, \
                     tc.tile_pool(name="qkstM", bufs=4) as qspM, \
                     tc.tile_pool(name="qkpsM", bufs=1,
                                  space="PSUM") as qpsM, \
                     tc.tile_pool(name="p2aM", bufs=2) as apM, \
                     tc.tile_pool(name="p2aMps", bufs=1,
                                  space="PSUM") as apsM, \
                     tc.tile_pool(name="fpsM", bufs=1,
                                  space="PSUM") as fpsM, \
                     tc.tile_pool(name="gpM", bufs=2) as gpM, \
                     tc.tile_pool(name="outM", bufs=3) as outM:
                    nc.vector.memset(zt[:, :, :, 0:1], 0.0)
                    nc.vector.memset(zt[:, :, :, W + 1:W + 2], 0.0)

                    wqk0M = qwpM.tile([C, 2 * CO], bf16, name="wqk0M",
                                      tag="wqk0")
                    nc.sync.dma_start(wqk0M[:], wqk[0])

                    # b1 side-work schedule: iteration -> items
                    side = {k: [] for k in range(NCH + 1)}
                    for ck in range(16):
                        side[ck // 3].append(("qk0", ck))
                    side[6] = [("qks", 1), ("qks", 2)]
                    side[7] = [("qks", 3)]
                    side[8] = [("ar", None)]

                    for k in range(NCH + 1):
                        if k < NCH:
                            chunk_2a(0, k, apM, apsM)
                            chunk_ln2(0, k, lpM, lpsM)
                        for kind, arg in side[k]:
                            if kind == "qk0":
                                qk_chunk(1, 0, 1, arg, wqk0M[:], qpsM,
                                         qspM, gramM[:], nmmM)
                            elif kind == "qks":
                                qk_scale_tail(1, arg, qwpM, qpsM, qspM,
                                              gramM[:], nmmM)
                            elif kind == "ar":
                                gram_done(1, gramM[:])
                                allreduce(1)
                            elif kind == "sm1":
                                pass
                        if k >= 2 and k - 2 < NFF:
                            chunk_ffn(0, k - 2, fpsM, apsM, gpM, outM)

            with tc.tile_pool(name="smx1", bufs=1) as sp1, \
                 tc.tile_pool(name="smxps1", bufs=2, space="PSUM") as sps1:
                softmax(1, sp1,
                        lambda: sps1.tile([CO, CO], f32, name="m1b",
                                          tag="m")[:])
                build_K(1,
                        lambda: sps1.tile([C, C], f32, name="kp1b",
                                          tag="kp")[:],
                        lambda: sps1.tile([C, 1], f32, name="bp1b",
                                          tag="bp")[:])

            # ==========================================================
            # b1 main loop
            # ==========================================================
            with tc.tile_pool(name="ln2", bufs=2) as lp2, \
                 tc.tile_pool(name="ln2ps", bufs=1, space="PSUM") as lps2, \
                 tc.tile_pool(name="p2a", bufs=2) as ap_, \
                 tc.tile_pool(name="p2aps", bufs=2, space="PSUM") as aps, \
                 tc.tile_pool(name="fps", bufs=1, space="PSUM") as fps, \
                 tc.tile_pool(name="gp", bufs=2) as gp, \
                 tc.tile_pool(name="outp", bufs=3) as outp:
                for k in range(NCH + 1):
                    if k < NCH:
                        chunk_2a(1, k, ap_, aps)
                        chunk_ln2(1, k, lp2, lps2)
                    if k >= 2 and k - 2 < NFF:
                        chunk_ffn(1, k - 2, fps, aps, gp, outp)

    nc.compile()
    return nc


# ---------------------------------------------------------------------------
# host side
# ---------------------------------------------------------------------------

def _prep_inputs(inputs, nrows, n_cores):
    import ml_dtypes
    bf = ml_dtypes.bfloat16
    H = nrows * n_cores
    x = np.asarray(inputs["x"], np.float32)
    n1w = np.asarray(inputs["n1w"], np.float32)
    n1b = np.asarray(inputs["n1b"], np.float32)
    n2w = np.asarray(inputs["n2w"], np.float32)
    n2b = np.asarray(inputs["n2b"], np.float32)

    wqk_taps = np.zeros((NTAPS, C, 2 * CO), np.float32)
    bqk = np.zeros((2 * CO, NS), np.float32)
    ti = 0
    for si, r in enumerate(SCALES):
        wqkw = np.asarray(inputs[f"wqk{si}"], np.float32)  # [64,128,r,r]
        wqkf = wqkw * n1w[None, :, None, None]
        bqk[:, si] = np.einsum("ocyx,c->o", wqkw, n1b)
        for dy in range(r):
            for dx in range(r):
                wqk_taps[ti] = wqkf[:, :, dy, dx].T
                ti += 1

    wv_cat = np.concatenate([np.asarray(inputs[f"wv{i}"],
                                        np.float32)[:, :, 0, 0]
                             for i in range(NS)], axis=0)      # [vch, in]
    bv_cat = np.concatenate([np.asarray(inputs[f"bv{i}"], np.float32)
                             for i in range(NS)])
    bv_all = (wv_cat @ n1b + bv_cat).astype(np.float32)
    wvn = (wv_cat * n1w[None, :])                              # [vch, in]

    wpj_t = np.asarray(inputs["wproj"], np.float32)[:, :, 0, 0].T.copy()

    wpi = np.asarray(inputs["wpi"], np.float32)[:, :, 0, 0]    # [680,128]
    wdw = np.asarray(inputs["wdw"], np.float32)[:, 0]          # [680,3,3]
    wf = np.zeros((9, C, 2 * HID), np.float32)
    for ti in range(9):
        dy, dx = divmod(ti, 3)
        wf[ti] = (wpi * wdw[:, dy, dx][:, None]).T             # [128,680]

    wpo = np.asarray(inputs["wpo"], np.float32)[:, :, 0, 0]    # [128,340]
    wpo_p = np.zeros((C, 3, C), np.float32)
    for p in range(3):
        mh = min(C, HID - C * p)
        wpo_p[:mh, p, :] = wpo[:, C * p:C * p + mh].T

    tv = np.zeros((1, NU), np.float32)
    for si in range(NS):
        for b in range(B):
            tv[0, b * NS + si] = float(
                np.asarray(inputs[f"t{si}"]).reshape(-1)[0])
    eye = np.eye(CO, dtype=np.float32)

    shared = {
        "wqk": wqk_taps.astype(bf), "bqk": bqk,
        "wvn": wvn.astype(bf), "bv": bv_all.reshape(C, 1).astype(bf),
        "wpj": wpj_t.astype(bf),
        "n1w": n1w.reshape(C, 1), "n1b": n1b.reshape(C, 1),
        "n2w": n2w.reshape(C, 1), "n2b": n2b.reshape(C, 1),
        "wf": wf.astype(bf), "wpo": wpo_p.astype(bf),
        "tvec": tv, "eye": eye,
        "ones": np.ones((C, C), np.float32).astype(bf),
    }

    in_maps = []
    for i in range(n_cores):
        r0 = nrows * i
        slab = np.zeros((B, C, nrows + 2, W), np.float32)
        lo, hi = r0 - 1, r0 + nrows + 1
        slo, shi = max(lo, 0), min(hi, H)
        slab[:, :, slo - lo:shi - lo, :] = x[:, :, slo:shi, :]
        m = {"xs": slab.astype(bf),
             "hmask": np.array([[1.0 if i > 0 else 0.0,
                                 1.0 if i < n_cores - 1 else 0.0]],
                               np.float32)}
        m.update(shared)
        in_maps.append(m)
    return in_maps


def _run(nrows, n_cores, in_maps, trace=False):
    from concourse.bass_utils import run_bass_kernel_spmd
    key = (nrows, n_cores)
    if key not in _CACHE:
        _CACHE[key] = _build(nrows, n_cores)
    nc = _CACHE[key]
    return run_bass_kernel_spmd(nc, in_maps, core_ids=list(range(n_cores)),
                                trace=trace)


def run_sharded(inputs, nrows=32, n_cores=8, trace=False):
    in_maps = _prep_inputs(inputs, nrows, n_cores)
    res = _run(nrows, n_cores, in_maps, trace=trace)
    H = nrows * n_cores
    out = np.zeros((B, C, H, W), np.float32)
    for i in range(n_cores):
        out[:, :, nrows * i:nrows * (i + 1), :] = res.results[i]["out"]
    return out, res


def kernel(**inputs):
    out, _ = run_sharded(inputs, nrows=32, n_cores=8)
    return out
